# revision 23
# baseline (speedup 1.0000x reference)
"""DeepseekV3 MLA flash-attention prefill kernel for 8 Trainium2 NeuronCores.

Sharding (SPMD, one program for all 8 cores):
  Stage A (sequence-parallel): core c owns 256 seq rows. Inputs arrive as a
    few large packed DMAs (x, then kv-weight wave, then q-weight wave); each
    wave accumulates into bank-exclusive PSUM groups. The kv AllGather fires
    as soon as the kv wave drains; the q AllGather is split in two 6-chunk
    halves of RAW (unnormalized) qa -- the rms scale row is gathered with the
    second half and applied post-projection, so the first gather fires at
    half-wave-1 end.
  Stage B (head-parallel): core c owns heads {2c, 2c+1}. K^T/V from the kv
    gather; q projections accumulate the two gathered halves (the second
    gather hides under the first half's projection work). Causal attention in
    (k, q) layout, no max-subtraction, fully-masked k-blocks skipped,
    diagonal blocks masked by a vector mask-add (softmax scale pre-folded
    into Wqb host-side).
  Output: per-panel partial Wo products (only this core's 2 head-rows of Wo)
    are exchanged with one AllToAll per 512-row panel and reduced on-core in
    f32; earlier panels' exchanges hide under later (heavier) panels'
    attention. Host reassembles the 64-row shards.
"""

import sys

if '/opt/trn_rl_repo' not in sys.path:
    sys.path.insert(0, '/opt/trn_rl_repo')

import numpy as np
import ml_dtypes

import concourse.bass as bass
import concourse.mybir as mybir
import concourse.tile as tile
from concourse import bacc
from concourse.bass_utils import run_bass_kernel_spmd

f32 = mybir.dt.float32
f32r = mybir.dt.float32r
bf16 = mybir.dt.bfloat16
i32 = mybir.dt.int32
AF = mybir.ActivationFunctionType
ALU = mybir.AluOpType

NC_ = 8            # cores
S = 2048           # sequence
HID = 2048
QLR = 1536         # q lora rank
KVLR = 512         # kv lora rank
ROPE = 64
NOPE = 128
VD = 128
NH = 16
HPC = NH // NC_    # heads per core = 2
SL = S // NC_      # rows per core = 256
PANEL = 512        # q panel width
NPANEL = S // PANEL
NKB = S // 128     # 16 k blocks
QCH = QLR // 128   # 12
QHALF = QCH // 2   # 6
KCH = KVLR // 128  # 4
HCH = HID // 128   # 16
KVW = KVLR + ROPE  # 576 = kv wave width
SHARD = PANEL // NC_  # 64 rows per core per panel
THETA = 10000.0
SM_SCALE = float((NOPE + ROPE) ** -0.5)
PI = float(np.pi)
NEG = -1e30

DT = bf16

_CACHE = {}


def _range_reduce_sin(nc, pool, src_ap, P, W, bias, name, res_pool=None, res_dt=f32, tagw=""):
    """sin(src + bias) with range reduction to [-pi, pi]. src may be PSUM."""
    t0 = pool.tile([P, W], f32, name=f"{name}_t0", tag=f"rr0{tagw}", bufs=1)
    ti = pool.tile([P, W], i32, name=f"{name}_ti", tag=f"rr1{tagw}", bufs=1)
    tf = pool.tile([P, W], f32, name=f"{name}_tf", tag=f"rr2{tagw}", bufs=1)
    arg = pool.tile([P, W], f32, name=f"{name}_arg", tag=f"rr3{tagw}", bufs=1)
    res = (res_pool or pool).tile([P, W], res_dt, name=f"{name}_sin", tag=f"res_{name}", bufs=1)
    nc.vector.tensor_scalar(out=t0[:], in0=src_ap, scalar1=bias, scalar2=None, op0=ALU.add)
    nc.vector.tensor_scalar(out=tf[:], in0=t0[:], scalar1=1.0 / (2 * PI), scalar2=None, op0=ALU.mult)
    nc.vector.tensor_copy(ti[:], tf[:])
    nc.vector.tensor_copy(tf[:], ti[:])
    nc.vector.scalar_tensor_tensor(out=arg[:], in0=tf[:], scalar=-2 * PI, in1=t0[:], op0=ALU.mult, op1=ALU.add)
    nc.scalar.activation(res[:], arg[:], AF.Sin)
    return res


def build_program(dt):
    nc = bacc.Bacc("TRN2", target_bir_lowering=False, debug=False, num_devices=NC_)

    def din(name, shape):
        return nc.dram_tensor(name, shape, dt, kind="ExternalInput")

    # ---- external I/O (per-core data, packed for large-row DMAs) ----
    x_p = din("x_p", [128, HCH * SL])            # hc-major packed X^T
    pos = nc.dram_tensor("pos", [1, SL], f32, kind="ExternalInput")
    pos_all = nc.dram_tensor("pos_all", [1, S], f32, kind="ExternalInput")
    wakv_p = din("wakv_p", [128, HCH * KVW])     # [Wkva(kv)|Wkva(pe,deint)] per hc
    waq_p = din("waq_p", [128, HCH * QLR])       # Wqa per hc
    wqb_p = din("wqb_p", [128, QCH * HPC * 256])  # [nope|pe_d|rot]*SM per head, per l
    wkk_p = din("wkk_p", [128, KCH * HPC * NOPE])
    wkv_p = din("wkv_p", [128, KCH * HPC * VD])
    wo_p = din("wo_p", [128, HPC * HID])         # Wo rows for this core's heads
    mask_in = din("mask", [128, 4 * PANEL])      # diag masks j=0..3 (0 / -1e30)
    ones_col = din("ones_col", [128, 1])
    ones_row = nc.dram_tensor("ones_row", [1, 128], f32, kind="ExternalInput")
    invf_col = nc.dram_tensor("invf_col", [ROPE, 1], f32, kind="ExternalInput")
    out_loc = nc.dram_tensor("out_loc", [NPANEL * SHARD, HID], f32, kind="ExternalOutput")

    with tile.TileContext(nc) as tc:
        with tc.tile_pool(name="dram", bufs=1, space="DRAM") as dpool, \
             tc.tile_pool(name="persist", bufs=1) as rp:
            ag_in_kv = dpool.tile([KVW, SL], dt)
            ag_out_kv = dpool.tile([NC_ * KVW, SL], dt, addr_space="Shared")
            ag_in_qa = dpool.tile([QHALF * 128, SL], dt)
            ag_out_qa = dpool.tile([NC_ * QHALF * 128, SL], dt, addr_space="Shared")
            ag_in_qb = dpool.tile([QHALF * 128 + 1, SL], dt)
            ag_out_qb = dpool.tile([NC_ * (QHALF * 128 + 1), SL], dt, addr_space="Shared")
            a2a_in = [dpool.tile([PANEL, HID], dt, name=f"a2a_in{p}") for p in range(NPANEL)]
            a2a_out = [dpool.tile([PANEL, HID], dt, name=f"a2a_out{p}") for p in range(NPANEL)]

            # ---- constants ----
            ocol = rp.tile([128, 1], dt)
            orow = rp.tile([1, 128], f32r)
            orow_bf = rp.tile([1, 128], dt)
            invc_t = rp.tile([ROPE, 1], f32)
            nc.sync.dma_start(out=ocol[:], in_=ones_col[:])
            nc.sync.dma_start(out=orow[:], in_=ones_row[:].bitcast(f32r))
            nc.sync.dma_start(out=invc_t[:], in_=invf_col[:])
            nc.vector.tensor_copy(orow_bf[:], orow[:].bitcast(f32))

            # stage B weight tiles (DMAs issued after the kv wave, below)
            mask_sb = rp.tile([128, 4 * PANEL], dt, name="mask_sb")
            wqb_all = rp.tile([128, QCH * HPC * 256], dt, name="wqb_all")
            wkk_all = rp.tile([128, KCH * HPC * NOPE], dt, name="wkk_all")
            wkv_all = rp.tile([128, KCH * HPC * VD], dt, name="wkv_all")
            wo_all = rp.tile([128, HPC * HID], dt, name="wo_all")

            def wqb_t(l):
                return wqb_all[:, 512 * l:512 * (l + 1)]

            def wkk_t(l):
                return wkk_all[:, 256 * l:256 * (l + 1)]

            def wkv_t(l):
                return wkv_all[:, 256 * l:256 * (l + 1)]

            def wo_sb(h):
                return wo_all[:, HID * h:HID * (h + 1)]

            sin_all = None
            cos_all = None

            # ================= Stage A =================
            with tc.tile_pool(name="sa_in", bufs=1) as sap, \
                 tc.tile_pool(name="sa_tmp", bufs=2) as tp, \
                 tc.tile_pool(name="sa_ps", bufs=6, space="PSUM") as accp, \
                 tc.tile_pool(name="sa_ps1", bufs=1, space="PSUM") as pp1:

                # packed input streams on the sync queue, priority order
                x_all = sap.tile([128, HCH * SL], dt, name="x_all")
                nc.sync.dma_start(out=x_all[:], in_=x_p[:])
                wakv_all = sap.tile([128, HCH * KVW], dt, name="wakv_all")
                nc.sync.dma_start(out=wakv_all[:], in_=wakv_p[:])
                waq_all = sap.tile([128, HCH * QLR], dt, name="waq_all")
                for qq in range(4):
                    w = HCH * QLR // 4
                    nc.sync.dma_start(out=waq_all[:, w * qq:w * (qq + 1)],
                                      in_=waq_p[:, w * qq:w * (qq + 1)])

                def xt(hc):
                    return x_all[:, SL * hc:SL * (hc + 1)]

                pos_all_t = tp.tile([1, S], f32r, name="pos_all_t", tag="posa", bufs=1)
                pos_t = tp.tile([1, SL], f32r, name="pos_t", tag="poso", bufs=1)
                nc.sync.dma_start(out=pos_all_t[:], in_=pos_all[:].bitcast(f32r))
                nc.sync.dma_start(out=pos_t[:], in_=pos[:].bitcast(f32r))
                emb_all = tp.tile([ROPE, S], f32, name="emb_all", tag="emba", bufs=1)

                # rope angle tables via K=1 outer products (one PSUM bank per
                # accumulation group -- matmul start zeroes a whole bank)
                for j in range(S // SL):
                    tb = accp.tile([128, SL], f32, name=f"tb_all{j}", tag="acc", bufs=6)
                    nc.tensor.matmul(tb[0:ROPE, :], orow[0:1, 0:ROPE],
                                     pos_all_t[:, SL * j:SL * (j + 1)], start=True, stop=True)
                    nc.vector.tensor_scalar(out=emb_all[:, SL * j:SL * (j + 1)],
                                            in0=tb[0:ROPE, :], scalar1=invc_t[:],
                                            scalar2=None, op0=ALU.mult)
                tb_own = accp.tile([128, SL], f32, name="tb_own", tag="acc", bufs=6)
                nc.tensor.matmul(tb_own[0:ROPE, 0:SL], orow[0:1, 0:ROPE], pos_t[:],
                                 start=True, stop=True)
                emb_own = tp.tile([ROPE, SL], f32, name="emb_own", tag="emb_own", bufs=1)
                nc.vector.tensor_scalar(out=emb_own[:], in0=tb_own[0:ROPE, 0:SL],
                                        scalar1=invc_t[:], scalar2=None, op0=ALU.mult)

                sin_all = _range_reduce_sin(nc, tp, emb_all[:], ROPE, S, 0.0, "sa",
                                            res_pool=rp, res_dt=dt, tagw="w")
                cos_all = _range_reduce_sin(nc, tp, emb_all[:], ROPE, S, PI / 2, "ca",
                                            res_pool=rp, res_dt=dt, tagw="w")
                sin_own = _range_reduce_sin(nc, tp, emb_own[:], ROPE, SL, 0.0, "so")
                cos_own = _range_reduce_sin(nc, tp, emb_own[:], ROPE, SL, PI / 2, "co")

                # ---- kv wave: chunks c0..c3 + pe accumulate over all hc ----
                acc_kv = [accp.tile([128, SL], f32, name=f"acc_kv{c}", tag="acc", bufs=6)
                          for c in range(KCH)]
                acc_pe = accp.tile([128, SL], f32, name="acc_pe", tag="acc", bufs=6)
                for hc in range(HCH):
                    st = (hc == 0)
                    sp = (hc == HCH - 1)
                    for c in range(KCH):
                        nc.tensor.matmul(acc_kv[c][:],
                                         wakv_all[:, KVW * hc + 128 * c:KVW * hc + 128 * (c + 1)],
                                         xt(hc), start=st, stop=sp)
                    nc.tensor.matmul(acc_pe[0:ROPE, :],
                                     wakv_all[:, KVW * hc + KVLR:KVW * hc + KVW],
                                     xt(hc), start=st, stop=sp)

                # kv ssq + rms scale
                ssq_kv = pp1.tile([1, SL], f32, name="ssq_kv", tag="ssq", bufs=1)
                sqs = []
                for c in range(KCH):
                    sq = tp.tile([128, SL], dt, name=f"sqk{c}", tag="sq", bufs=4)
                    nc.scalar.activation(sq[:], acc_kv[c][:], AF.Square)
                    sqs.append(sq)
                for c in range(KCH):
                    nc.tensor.matmul(ssq_kv[:], ocol[:], sqs[c][:],
                                     start=(c == 0), stop=(c == KCH - 1))
                ms_kv = tp.tile([1, SL], f32, name="ms_kv", tag="ms", bufs=2)
                nc.scalar.activation(ms_kv[:], ssq_kv[:], AF.Sqrt, scale=1.0 / KVLR)
                rkv = tp.tile([1, SL], f32, name="rkv", tag="rr", bufs=2)
                nc.vector.reciprocal_approx_fast(out=rkv[:], in_=ms_kv[:])
                rkvr = tp.tile([1, SL], f32r, name="rkvr", tag="rrr", bufs=2)
                with nc.allow_low_precision(reason="f32r rounding of rms scale"):
                    nc.vector.tensor_copy(rkvr[:], rkv[:])
                bc_kv = pp1.tile([128, SL], f32, name="bc_kv", tag="bc", bufs=1)
                nc.tensor.matmul(bc_kv[:], orow[:], rkvr[:], start=True, stop=True)
                bckv_sb = tp.tile([128, SL], f32, name="bckv_sb", tag="bc_sb", bufs=2)
                nc.scalar.activation(bckv_sb[:], bc_kv[:], AF.Copy)

                # k_pe rope
                krot = tp.tile([ROPE, SL], f32, name="krot", tag="krot", bufs=1)
                nc.vector.tensor_scalar(out=krot[0:32, :], in0=acc_pe[32:64, :],
                                        scalar1=-1.0, scalar2=None, op0=ALU.mult)
                nc.vector.tensor_copy(krot[32:64, :], acc_pe[0:32, :])
                kro = tp.tile([ROPE, SL], f32, name="kro", tag="kro", bufs=1)
                nc.vector.tensor_mul(kro[:], acc_pe[0:ROPE, :], cos_own[:])
                krs = tp.tile([ROPE, SL], f32, name="krs", tag="krs", bufs=1)
                nc.vector.tensor_mul(krs[:], krot[:], sin_own[:])
                kfin = tp.tile([ROPE, SL], dt, name="kfin", tag="kfin", bufs=1)
                nc.vector.tensor_add(kfin[:], kro[:], krs[:])
                nc.scalar.dma_start(out=ag_in_kv[KVLR:KVLR + ROPE, :], in_=kfin[:])

                for c in range(KCH):
                    sc = tp.tile([128, SL], dt, name=f"sck{c}", tag="sc", bufs=4)
                    nc.vector.tensor_mul(sc[:], acc_kv[c][:], bckv_sb[:])
                    nc.scalar.dma_start(out=ag_in_kv[128 * c:128 * (c + 1), :], in_=sc[:])

                nc.gpsimd.collective_compute(
                    "AllGather", ALU.bypass,
                    replica_groups=[list(range(NC_))],
                    ins=[ag_in_kv[:]], outs=[ag_out_kv[:]],
                )

                # stage B weights now that the stage-A input stream has drained
                nc.scalar.dma_start(out=wkk_all[:], in_=wkk_p[:])
                nc.scalar.dma_start(out=wkv_all[:], in_=wkv_p[:])
                nc.scalar.dma_start(out=wqb_all[:], in_=wqb_p[:])
                nc.scalar.dma_start(out=mask_sb[:], in_=mask_in[:])
                nc.scalar.dma_start(out=wo_all[:], in_=wo_p[:])

                # ---- q wave in two halves of 6 chunks, gathered RAW ----
                ssq_q = pp1.tile([1, SL], f32, name="ssq_q", tag="ssq", bufs=1)
                sqq = []
                acc_q1 = [accp.tile([128, SL], f32, name=f"acc_q1_{c}", tag="acc", bufs=6)
                          for c in range(QHALF)]
                for hc in range(HCH):
                    st = (hc == 0)
                    sp = (hc == HCH - 1)
                    for c in range(QHALF):
                        nc.tensor.matmul(acc_q1[c][:],
                                         waq_all[:, QLR * hc + 128 * c:QLR * hc + 128 * (c + 1)],
                                         xt(hc), start=st, stop=sp)
                for c in range(QHALF):
                    raw = tp.tile([128, SL], dt, name=f"rawqa{c}", tag="sc", bufs=4)
                    nc.vector.tensor_copy(raw[:], acc_q1[c][:])
                    nc.scalar.dma_start(out=ag_in_qa[128 * c:128 * (c + 1), :], in_=raw[:])
                    sq = tp.tile([128, SL], dt, name=f"sqq{c}", tag="sq", bufs=4)
                    nc.scalar.activation(sq[:], acc_q1[c][:], AF.Square)
                    sqq.append(sq)
                nc.gpsimd.collective_compute(
                    "AllGather", ALU.bypass,
                    replica_groups=[list(range(NC_))],
                    ins=[ag_in_qa[:]], outs=[ag_out_qa[:]],
                )
                for c in range(QHALF):
                    nc.tensor.matmul(ssq_q[:], ocol[:], sqq[c][:],
                                     start=(c == 0), stop=False)

                acc_q2 = [accp.tile([128, SL], f32, name=f"acc_q2_{c}", tag="acc", bufs=6)
                          for c in range(QHALF)]
                for hc in range(HCH):
                    st = (hc == 0)
                    sp = (hc == HCH - 1)
                    for c in range(QHALF):
                        cc = c + QHALF
                        nc.tensor.matmul(acc_q2[c][:],
                                         waq_all[:, QLR * hc + 128 * cc:QLR * hc + 128 * (cc + 1)],
                                         xt(hc), start=st, stop=sp)
                for c in range(QHALF):
                    raw = tp.tile([128, SL], dt, name=f"rawqb{c}", tag="sc", bufs=4)
                    nc.vector.tensor_copy(raw[:], acc_q2[c][:])
                    nc.scalar.dma_start(out=ag_in_qb[128 * c:128 * (c + 1), :], in_=raw[:])
                    sq = tp.tile([128, SL], dt, name=f"sqq{c + QHALF}", tag="sq", bufs=4)
                    nc.scalar.activation(sq[:], acc_q2[c][:], AF.Square)
                    sqq.append(sq)
                for c in range(QHALF):
                    nc.tensor.matmul(ssq_q[:], ocol[:], sqq[c + QHALF][:],
                                     start=False, stop=(c == QHALF - 1))
                ms_q = tp.tile([1, SL], f32, name="ms_q", tag="ms", bufs=2)
                nc.scalar.activation(ms_q[:], ssq_q[:], AF.Sqrt, scale=1.0 / QLR)
                rq = tp.tile([1, SL], f32, name="rq", tag="rr", bufs=2)
                nc.vector.reciprocal_approx_fast(out=rq[:], in_=ms_q[:])
                r_bf = tp.tile([1, SL], dt, name="r_bf", tag="rbf", bufs=1)
                nc.vector.tensor_copy(r_bf[:], rq[:])
                nc.scalar.dma_start(out=ag_in_qb[QHALF * 128:QHALF * 128 + 1, :], in_=r_bf[:])
                nc.gpsimd.collective_compute(
                    "AllGather", ALU.bypass,
                    replica_groups=[list(range(NC_))],
                    ins=[ag_in_qb[:]], outs=[ag_out_qb[:]],
                )

            agkv_r = ag_out_kv.rearrange("(r c) q -> r c q", r=NC_)
            agqa_r = ag_out_qa.rearrange("(r c) q -> r c q", r=NC_)
            agqb_r = ag_out_qb.rearrange("(r c) q -> r c q", r=NC_)

            # ================= Stage B =================
            with tc.tile_pool(name="sb_res", bufs=1) as sbp, \
                 tc.tile_pool(name="sb_tmp", bufs=2) as tp, \
                 tc.tile_pool(name="sb_qa", bufs=2) as qap, \
                 tc.tile_pool(name="sb_pt", bufs=4) as ptp, \
                 tc.tile_pool(name="sb_mm", bufs=2, space="PSUM") as pmm, \
                 tc.tile_pool(name="sb_at", bufs=2, space="PSUM") as pat, \
                 tc.tile_pool(name="sb_ps1", bufs=1, space="PSUM") as pp1, \
                 tc.tile_pool(name="sb_wo", bufs=2, space="PSUM") as pwo:

                # K^T and V (both heads)
                kpe_g = sbp.tile([ROPE, S], dt, name="kpe_g")
                for r in range(NC_):
                    nc.sync.dma_start(out=kpe_g[:, SL * r:SL * (r + 1)],
                                      in_=agkv_r[r, KVLR:KVLR + ROPE, :])
                kT = [sbp.tile([128, S], dt, name=f"kT{h}") for h in range(HPC)]
                v_t = [sbp.tile([128, HPC * VD], dt, name=f"v_t{kb}") for kb in range(NKB)]
                with tc.tile_pool(name="sb_ckv", bufs=1) as ckvp:
                    ckv_g = []
                    for j in range(KCH):
                        t = ckvp.tile([128, S], dt, name=f"ckv_g{j}")
                        for r in range(NC_):
                            nc.sync.dma_start(out=t[:, SL * r:SL * (r + 1)],
                                              in_=agkv_r[r, 128 * j:128 * (j + 1), :])
                        ckv_g.append(t)
                    ei = 0
                    for h in range(HPC):
                        for kc in range(S // 512):
                            ps = pmm.tile([128, 512], f32, name=f"kt_ps{h}_{kc}", tag="mm", bufs=2)
                            for l in range(KCH):
                                nc.tensor.matmul(ps[:], wkk_t(l)[:, NOPE * h:NOPE * (h + 1)],
                                                 ckv_g[l][:, 512 * kc:512 * (kc + 1)],
                                                 start=(l == 0), stop=(l == KCH - 1))
                            if ei % 2 == 0:
                                nc.vector.tensor_copy(kT[h][:, 512 * kc:512 * (kc + 1)], ps[:])
                            else:
                                nc.scalar.activation(kT[h][:, 512 * kc:512 * (kc + 1)], ps[:], AF.Copy)
                            ei += 1
                    for kb in range(NKB):
                        ps = pmm.tile([128, HPC * VD], f32, name=f"v_ps{kb}", tag="mm", bufs=2)
                        for l in range(KCH):
                            nc.tensor.matmul(ps[:], ckv_g[l][:, 128 * kb:128 * (kb + 1)],
                                             wkv_t(l), start=(l == 0), stop=(l == KCH - 1))
                        if kb % 2 == 0:
                            nc.vector.tensor_copy(v_t[kb][:], ps[:])
                        else:
                            nc.scalar.activation(v_t[kb][:], ps[:], AF.Copy)

                # ---- q projections: half a (chunks 0..5) ----
                qa_pa = {}
                for p in range(NPANEL):
                    for l in range(QHALF):
                        t = qap.tile([128, PANEL], dt, name=f"qa_pa{p}_{l}", tag="qaa", bufs=8)
                        for r in range(2):
                            nc.sync.dma_start(out=t[:, SL * r:SL * (r + 1)],
                                              in_=agqa_r[2 * p + r, 128 * l:128 * (l + 1), :])
                        qa_pa[(p, l)] = t
                qn_a = {}
                qr_a = {}
                for p in range(NPANEL):
                    for h in range(HPC):
                        hcol = 256 * h
                        ps_qn = pmm.tile([128, PANEL], f32, name=f"qna_ps{h}_{p}", tag="mm", bufs=2)
                        for l in range(QHALF):
                            nc.tensor.matmul(ps_qn[:], wqb_t(l)[:, hcol:hcol + NOPE],
                                             qa_pa[(p, l)][:], start=(l == 0), stop=(l == QHALF - 1))
                        ps_qr = pmm.tile([128, PANEL], f32, name=f"qra_ps{h}_{p}", tag="mm", bufs=2)
                        for l in range(QHALF):
                            nc.tensor.matmul(ps_qr[:], wqb_t(l)[:, hcol + NOPE:hcol + 256],
                                             qa_pa[(p, l)][:], start=(l == 0), stop=(l == QHALF - 1))
                        tn = sbp.tile([128, PANEL], dt, name=f"qn_a{h}_{p}")
                        nc.scalar.activation(tn[:], ps_qn[:], AF.Copy)
                        qn_a[(h, p)] = tn
                        tr_lo = sbp.tile([ROPE, PANEL], dt, name=f"qr_alo{h}_{p}")
                        nc.vector.tensor_copy(tr_lo[:], ps_qr[0:ROPE, :])
                        tr_hi = sbp.tile([ROPE, PANEL], dt, name=f"qr_ahi{h}_{p}")
                        nc.vector.tensor_copy(tr_hi[:], ps_qr[ROPE:2 * ROPE, :])
                        qr_a[(h, p)] = (tr_lo, tr_hi)

                # ---- half b + rms scale + rope ----
                qa_pb = {}
                for p in range(NPANEL):
                    for l in range(QHALF):
                        t = qap.tile([128, PANEL], dt, name=f"qa_pb{p}_{l}", tag="qab", bufs=8)
                        for r in range(2):
                            nc.sync.dma_start(out=t[:, SL * r:SL * (r + 1)],
                                              in_=agqb_r[2 * p + r, 128 * l:128 * (l + 1), :])
                        qa_pb[(p, l)] = t
                qn_sb = {}
                qp_sb = {}
                for p in range(NPANEL):
                    qs = slice(PANEL * p, PANEL * (p + 1))
                    # per-panel rms scale row (gathered raw with half b)
                    rrow = tp.tile([1, PANEL], dt, name=f"rrow{p}", tag="rrow", bufs=2)
                    for r in range(2):
                        nc.sync.dma_start(out=rrow[0:1, SL * r:SL * (r + 1)],
                                          in_=agqb_r[2 * p + r, QHALF * 128:QHALF * 128 + 1, :])
                    rbc = pp1.tile([128, PANEL], f32, name=f"rbc{p}", tag="bcb", bufs=1)
                    nc.tensor.matmul(rbc[:], orow_bf[:], rrow[:], start=True, stop=True)
                    rbc_sb = tp.tile([128, PANEL], f32, name=f"rbc_sb{p}", tag="bc_sb", bufs=2)
                    nc.scalar.activation(rbc_sb[:], rbc[:], AF.Copy)
                    for h in range(HPC):
                        hcol = 256 * h
                        ps_qn = pmm.tile([128, PANEL], f32, name=f"qnb_ps{h}_{p}", tag="mm", bufs=2)
                        for l in range(QHALF):
                            nc.tensor.matmul(ps_qn[:], wqb_t(l + QHALF)[:, hcol:hcol + NOPE],
                                             qa_pb[(p, l)][:], start=(l == 0), stop=(l == QHALF - 1))
                        ps_qr = pmm.tile([128, PANEL], f32, name=f"qrb_ps{h}_{p}", tag="mm", bufs=2)
                        for l in range(QHALF):
                            nc.tensor.matmul(ps_qr[:], wqb_t(l + QHALF)[:, hcol + NOPE:hcol + 256],
                                             qa_pb[(p, l)][:], start=(l == 0), stop=(l == QHALF - 1))
                        qsum = tp.tile([128, PANEL], f32, name=f"qsum{h}_{p}", tag="qsum", bufs=2)
                        nc.vector.tensor_add(qsum[:], qn_a[(h, p)][:], ps_qn[:])
                        qn = sbp.tile([128, PANEL], dt, name=f"qn_sb{h}_{p}")
                        nc.vector.tensor_mul(qn[:], qsum[:], rbc_sb[:])
                        qn_sb[(h, p)] = qn
                        tr_lo, tr_hi = qr_a[(h, p)]
                        s_lo = tp.tile([ROPE, PANEL], f32, name=f"slo{h}_{p}", tag="slo", bufs=2)
                        nc.vector.tensor_add(s_lo[:], tr_lo[:], ps_qr[0:ROPE, :])
                        s_hi = tp.tile([ROPE, PANEL], f32, name=f"shi{h}_{p}", tag="shi", bufs=2)
                        nc.vector.tensor_add(s_hi[:], tr_hi[:], ps_qr[ROPE:2 * ROPE, :])
                        qt1 = tp.tile([ROPE, PANEL], f32, name=f"qt1_{h}_{p}", tag="qt1", bufs=2)
                        nc.vector.tensor_mul(qt1[:], s_lo[:], cos_all[:, qs])
                        qt2 = tp.tile([ROPE, PANEL], f32, name=f"qt2_{h}_{p}", tag="qt2", bufs=2)
                        nc.vector.tensor_mul(qt2[:], s_hi[:], sin_all[:, qs])
                        qpp = tp.tile([ROPE, PANEL], f32, name=f"qpp{h}_{p}", tag="qpp", bufs=2)
                        nc.vector.tensor_add(qpp[:], qt1[:], qt2[:])
                        qp = sbp.tile([ROPE, PANEL], dt, name=f"qp_sb{h}_{p}")
                        nc.vector.tensor_mul(qp[:], qpp[:], rbc_sb[0:ROPE, :])
                        qp_sb[(h, p)] = qp

                # ---- attention + per-panel Wo partials + AllToAll + reduce ----
                with tc.tile_pool(name="sb_red", bufs=1) as redp:
                    def reduce_panel(p):
                        rbs = []
                        for r in range(NC_):
                            t = redp.tile([SHARD, HID], dt, name=f"rb{p}_{r}", tag="rb", bufs=4)
                            nc.sync.dma_start(out=t[:], in_=a2a_out[p][SHARD * r:SHARD * (r + 1), :])
                            rbs.append(t)
                        acc = redp.tile([SHARD, HID], f32, name=f"racc{p}_0", tag="racc", bufs=2)
                        nc.vector.tensor_add(acc[:], rbs[0][:], rbs[1][:])
                        for r in range(2, NC_):
                            nxt = redp.tile([SHARD, HID], f32, name=f"racc{p}_{r}", tag="racc", bufs=2)
                            nc.vector.tensor_add(nxt[:], acc[:], rbs[r][:])
                            acc = nxt
                        nc.scalar.dma_start(out=out_loc[SHARD * p:SHARD * (p + 1), :], in_=acc[:])

                    for p in range(NPANEL):
                        at_ps = {}
                        for h in range(HPC):
                            nkb = 4 * (p + 1)
                            ps_at = pat.tile([128, PANEL], f32, name=f"at_ps{h}_{p}", tag="at", bufs=2)
                            ps_sum = pp1.tile([1, PANEL], f32, name=f"sum_ps{h}_{p}", tag="sum", bufs=1)
                            pts = {}

                            def consume(kb, nkb=nkb, ps_at=ps_at, ps_sum=ps_sum, pts=pts, h=h):
                                nc.tensor.matmul(ps_at[:], v_t[kb][:, VD * h:VD * (h + 1)], pts[kb][:],
                                                 start=(kb == 0), stop=(kb == nkb - 1))
                                nc.tensor.matmul(ps_sum[:], ocol[:], pts[kb][:],
                                                 start=(kb == 0), stop=(kb == nkb - 1))

                            for kb in range(nkb):
                                ps_sc = pmm.tile([128, PANEL], f32, name=f"sc_ps{h}_{p}_{kb}",
                                                 tag="mm", bufs=2)
                                nc.tensor.matmul(ps_sc[:], kT[h][:, 128 * kb:128 * (kb + 1)],
                                                 qn_sb[(h, p)][:], start=True, stop=False)
                                nc.tensor.matmul(ps_sc[:], kpe_g[:, 128 * kb:128 * (kb + 1)],
                                                 qp_sb[(h, p)][:], start=False, stop=True)
                                pt = ptp.tile([128, PANEL], dt, name=f"pt{h}_{p}_{kb}", tag="pt", bufs=4)
                                if kb >= 4 * p:
                                    j = kb - 4 * p
                                    msc = tp.tile([128, PANEL], f32, name=f"msc{h}_{p}_{kb}",
                                                  tag="msc", bufs=2)
                                    nc.vector.tensor_add(msc[:], ps_sc[:],
                                                         mask_sb[:, PANEL * j:PANEL * (j + 1)])
                                    nc.scalar.activation(pt[:], msc[:], AF.Exp)
                                else:
                                    nc.scalar.activation(pt[:], ps_sc[:], AF.Exp)
                                pts[kb] = pt
                                if kb > 0:
                                    consume(kb - 1)
                            consume(nkb - 1)
                            rec = tp.tile([1, PANEL], f32, name=f"rec{h}_{p}", tag="rec", bufs=2)
                            nc.vector.reciprocal_approx_fast(out=rec[:], in_=ps_sum[:])
                            recr = tp.tile([1, PANEL], f32r, name=f"recr{h}_{p}", tag="recr", bufs=2)
                            with nc.allow_low_precision(reason="f32r rounding of softmax recip"):
                                nc.vector.tensor_copy(recr[:], rec[:])
                            bc = pp1.tile([128, PANEL], f32, name=f"bc_ps{h}_{p}", tag="bcb", bufs=1)
                            nc.tensor.matmul(bc[:], orow[:], recr[:], start=True, stop=True)
                            bc_sb = tp.tile([128, PANEL], f32, name=f"bc_sb{h}_{p}", tag="bc_sb", bufs=2)
                            nc.scalar.activation(bc_sb[:], bc[:], AF.Copy)
                            at_p = tp.tile([128, PANEL], dt, name=f"at_p{h}_{p}", tag="at_p", bufs=3)
                            nc.vector.tensor_mul(at_p[:], ps_at[:], bc_sb[:])
                            at_ps[h] = at_p

                        # previous panel's exchange has landed by now: reduce it
                        if p > 0:
                            reduce_panel(p - 1)

                        # Wo partial for this panel (rows = panel's 4 seq blocks)
                        for sb in range(4):
                            ev = tp.tile([128, HID], dt, name=f"woev{p}_{sb}", tag="woev", bufs=2)
                            for n in range(4):
                                ps_o = pwo.tile([128, 512], f32, name=f"wo_ps{p}_{sb}_{n}",
                                                tag="wo", bufs=2)
                                for h in range(HPC):
                                    nc.tensor.matmul(ps_o[:], at_ps[h][:, 128 * sb:128 * (sb + 1)],
                                                     wo_sb(h)[:, 512 * n:512 * (n + 1)],
                                                     start=(h == 0), stop=(h == HPC - 1))
                                if sb % 2 == 0:
                                    nc.vector.tensor_copy(ev[:, 512 * n:512 * (n + 1)], ps_o[:])
                                else:
                                    nc.scalar.activation(ev[:, 512 * n:512 * (n + 1)], ps_o[:], AF.Copy)
                            nc.scalar.dma_start(out=a2a_in[p][128 * sb:128 * (sb + 1), :], in_=ev[:])
                        nc.gpsimd.collective_compute(
                            "AllToAll", ALU.bypass,
                            replica_groups=[list(range(NC_))],
                            ins=[a2a_in[p][:]], outs=[a2a_out[p][:]],
                        )
                    reduce_panel(NPANEL - 1)

    nc.compile()
    return nc


def _to_dt(a, dt):
    if dt == bf16:
        return np.ascontiguousarray(a.astype(ml_dtypes.bfloat16))
    return np.ascontiguousarray(a.astype(np.float32))


def _prepare_inputs(dt, hidden_states, position_ids, Wqa, qa_ln_w, Wqb, Wkva, kv_ln_w, Wkvb, Wo):
    perm = np.concatenate([np.arange(0, ROPE, 2), np.arange(1, ROPE, 2)])
    X = np.asarray(hidden_states, np.float32).reshape(S, HID)
    pos_f = np.ascontiguousarray(np.asarray(position_ids, np.float32).reshape(1, S))
    Wqa = np.asarray(Wqa, np.float32)
    Wkva = np.asarray(Wkva, np.float32)
    wa_kv = np.concatenate([Wkva[:, :KVLR], Wkva[:, KVLR:][:, perm]], axis=1)  # (2048, 576)
    wqb_base = np.asarray(Wqb, np.float32) * np.asarray(qa_ln_w, np.float32)[:, None]
    wkvb_base = np.asarray(Wkvb, np.float32) * np.asarray(kv_ln_w, np.float32)[:, None]
    Wo = np.asarray(Wo, np.float32)

    head_blocks = []
    for h in range(NH):
        cols = wqb_base[:, 192 * h:192 * (h + 1)] * SM_SCALE
        nope = cols[:, :NOPE]
        pe_d = cols[:, NOPE:][:, perm]
        rot = np.concatenate([-pe_d[:, 32:], pe_d[:, :32]], axis=1)
        head_blocks.append(np.concatenate([nope, pe_d, rot], axis=1))  # (1536, 256)
    k_blocks = [wkvb_base[:, 256 * h:256 * h + NOPE] for h in range(NH)]
    v_blocks = [wkvb_base[:, 256 * h + NOPE:256 * (h + 1)] for h in range(NH)]

    inv = (1.0 / (THETA ** (np.arange(0, ROPE, 2, dtype=np.float32) / ROPE))).astype(np.float32)
    invf_np = np.concatenate([inv, inv])

    # diagonal masks: block j, mask[r, col] = NEG where col < 128 j + r
    colsi = np.arange(PANEL)[None, :]
    rowsi = np.arange(128)[:, None]
    mask_np = np.concatenate(
        [np.where(colsi < 128 * j + rowsi, NEG, 0.0) for j in range(4)], axis=1
    ).astype(np.float32)

    def pack_rows(w):
        # [HCH*128, W] -> [128, HCH*W] packing (hc-major along free dim)
        ch = w.shape[0] // 128
        return np.concatenate([w[128 * k:128 * (k + 1), :] for k in range(ch)], axis=1)

    wa_kv_d = _to_dt(pack_rows(wa_kv), dt)
    wa_q_d = _to_dt(pack_rows(Wqa), dt)
    mask_d = _to_dt(mask_np, dt)
    ones_col_d = _to_dt(np.ones((128, 1), np.float32), dt)

    in_maps = []
    for c in range(NC_):
        rows_c = slice(SL * c, SL * (c + 1))
        wqb_c = np.concatenate([head_blocks[HPC * c + h] for h in range(HPC)], axis=1)
        wkk_c = np.concatenate([k_blocks[HPC * c + h] for h in range(HPC)], axis=1)
        wkv_c = np.concatenate([v_blocks[HPC * c + h] for h in range(HPC)], axis=1)
        wo_c = np.concatenate([Wo[VD * (HPC * c + h):VD * (HPC * c + h + 1), :]
                               for h in range(HPC)], axis=0)
        in_maps.append({
            "x_p": _to_dt(pack_rows(np.ascontiguousarray(X[rows_c, :].T)), dt),
            "pos": np.ascontiguousarray(pos_f[:, rows_c]),
            "pos_all": pos_f,
            "wakv_p": wa_kv_d,
            "waq_p": wa_q_d,
            "wqb_p": _to_dt(pack_rows(wqb_c), dt),
            "wkk_p": _to_dt(pack_rows(wkk_c), dt),
            "wkv_p": _to_dt(pack_rows(wkv_c), dt),
            "wo_p": _to_dt(pack_rows(wo_c), dt),
            "mask": mask_d,
            "ones_col": ones_col_d,
            "ones_row": np.ones((1, 128), np.float32),
            "invf_col": invf_np.reshape(ROPE, 1).copy(),
        })
    return in_maps


def run(inputs, trace=False, trace_cores=None, dt=None):
    dt = dt if dt is not None else DT
    key = ("nc", str(dt))
    if key not in _CACHE:
        _CACHE[key] = build_program(dt)
    nc = _CACHE[key]
    in_maps = _prepare_inputs(dt, **inputs)
    res = run_bass_kernel_spmd(nc, in_maps, list(range(NC_)), trace=trace,
                               trace_cores=trace_cores)
    # reassemble: panel p, core c holds global seq rows [512 p + 64 c, 512 p + 64 (c+1))
    out = np.empty((S, HID), np.float32)
    for c in range(NC_):
        o = res.results[c]["out_loc"]
        for p in range(NPANEL):
            out[PANEL * p + SHARD * c:PANEL * p + SHARD * (c + 1), :] = \
                o[SHARD * p:SHARD * (p + 1), :]
    return out.reshape(1, S, HID), res


def kernel(**inputs) -> np.ndarray:
    out, _ = run(inputs, trace=False)
    return out


# revision 24
# speedup vs baseline: 1.0669x; 1.0669x over previous
"""DeepseekV3 MLA flash-attention prefill kernel for 8 Trainium2 NeuronCores.

Sharding (SPMD, one program for all 8 cores):
  Stage A (sequence-parallel): core c owns 256 seq rows. Inputs arrive as a
    dependency-chained sequence of large packed DMAs (x || wa_kv, then the
    wa_q quarters, then stage-B weights) so early tiles are never delayed by
    later transfers interleaving on the same queue. Each weight wave
    accumulates into bank-exclusive PSUM groups. The kv AllGather fires right
    at the initial-barrier horizon; the q AllGather carries RAW (unnormalized)
    qa plus the rms scale row, applied post-projection in stage B.
  Stage B (head-parallel): core c owns heads {2c, 2c+1}. K^T/V from the kv
    gather. Causal attention in (k, q) layout, no max-subtraction,
    fully-masked k-blocks skipped, diagonal blocks masked by a vector
    mask-add (softmax scale pre-folded into Wqb host-side).
  Output: per-panel partial Wo products (only this core's 2 head-rows of Wo)
    are exchanged with one AllToAll per 512-row panel and reduced on-core in
    f32; earlier panels' exchanges hide under later (heavier) panels'
    attention. The last panel's exchange is split into two hid-halves so its
    first half's reduction overlaps the second half's transfer.
"""

import sys

if '/opt/trn_rl_repo' not in sys.path:
    sys.path.insert(0, '/opt/trn_rl_repo')

import numpy as np
import ml_dtypes

import concourse.bass as bass
import concourse.mybir as mybir
import concourse.tile as tile
from concourse import bacc
from concourse.bass_utils import run_bass_kernel_spmd

f32 = mybir.dt.float32
f32r = mybir.dt.float32r
bf16 = mybir.dt.bfloat16
i32 = mybir.dt.int32
AF = mybir.ActivationFunctionType
ALU = mybir.AluOpType

NC_ = 8            # cores
S = 2048           # sequence
HID = 2048
QLR = 1536         # q lora rank
KVLR = 512         # kv lora rank
ROPE = 64
NOPE = 128
VD = 128
NH = 16
HPC = NH // NC_    # heads per core = 2
SL = S // NC_      # rows per core = 256
PANEL = 512        # q panel width
NPANEL = S // PANEL
NKB = S // 128     # 16 k blocks
QCH = QLR // 128   # 12
QHALF = QCH // 2   # 6
KCH = KVLR // 128  # 4
HCH = HID // 128   # 16
KVW = KVLR + ROPE  # 576 = kv wave width
SHARD = PANEL // NC_  # 64 rows per core per panel
THETA = 10000.0
SM_SCALE = float((NOPE + ROPE) ** -0.5)
PI = float(np.pi)
NEG = -1e30

DT = bf16

_CACHE = {}


def _range_reduce_sin(nc, pool, src_ap, P, W, bias, name, res_pool=None, res_dt=f32, tagw=""):
    """sin(src + bias) with range reduction to [-pi, pi]. src may be PSUM."""
    t0 = pool.tile([P, W], f32, name=f"{name}_t0", tag=f"rr0{tagw}", bufs=1)
    ti = pool.tile([P, W], i32, name=f"{name}_ti", tag=f"rr1{tagw}", bufs=1)
    tf = pool.tile([P, W], f32, name=f"{name}_tf", tag=f"rr2{tagw}", bufs=1)
    arg = pool.tile([P, W], f32, name=f"{name}_arg", tag=f"rr3{tagw}", bufs=1)
    res = (res_pool or pool).tile([P, W], res_dt, name=f"{name}_sin", tag=f"res_{name}", bufs=1)
    nc.vector.tensor_scalar(out=t0[:], in0=src_ap, scalar1=bias, scalar2=None, op0=ALU.add)
    nc.vector.tensor_scalar(out=tf[:], in0=t0[:], scalar1=1.0 / (2 * PI), scalar2=None, op0=ALU.mult)
    nc.vector.tensor_copy(ti[:], tf[:])
    nc.vector.tensor_copy(tf[:], ti[:])
    nc.vector.scalar_tensor_tensor(out=arg[:], in0=tf[:], scalar=-2 * PI, in1=t0[:], op0=ALU.mult, op1=ALU.add)
    nc.scalar.activation(res[:], arg[:], AF.Sin)
    return res


def build_program(dt):
    nc = bacc.Bacc("TRN2", target_bir_lowering=False, debug=False, num_devices=NC_)

    def din(name, shape):
        return nc.dram_tensor(name, shape, dt, kind="ExternalInput")

    # ---- external I/O (per-core data, packed for large-row DMAs) ----
    x_p = din("x_p", [128, HCH * SL])            # hc-major packed X^T
    pos = nc.dram_tensor("pos", [1, SL], f32, kind="ExternalInput")
    pos_all = nc.dram_tensor("pos_all", [1, S], f32, kind="ExternalInput")
    wakv_p = din("wakv_p", [128, HCH * KVW])     # [Wkva(kv)|Wkva(pe,deint)] per hc
    waq_p = din("waq_p", [128, HCH * QLR])       # Wqa per hc
    wqb_p = din("wqb_p", [128, QCH * HPC * 256])  # [nope|pe_d|rot]*SM per head, per l
    wkk_p = din("wkk_p", [128, KCH * HPC * NOPE])
    wkv_p = din("wkv_p", [128, KCH * HPC * VD])
    wo_p = din("wo_p", [128, HPC * HID])         # Wo rows for this core's heads
    mask_in = din("mask", [128, 4 * PANEL])      # diag masks j=0..3 (0 / -1e30)
    ones_col = din("ones_col", [128, 1])
    ones_row = nc.dram_tensor("ones_row", [1, 128], f32, kind="ExternalInput")
    invf_col = nc.dram_tensor("invf_col", [ROPE, 1], f32, kind="ExternalInput")
    out_loc = nc.dram_tensor("out_loc", [NPANEL * SHARD, HID], f32, kind="ExternalOutput")

    QROWS = QCH * 128 + 1  # 12 raw chunks + rms scale row

    with tile.TileContext(nc) as tc:
        with tc.tile_pool(name="dram", bufs=1, space="DRAM") as dpool, \
             tc.tile_pool(name="persist", bufs=1) as rp:
            ag_in_kv = dpool.tile([KVW, SL], dt)
            ag_out_kv = dpool.tile([NC_ * KVW, SL], dt, addr_space="Shared")
            ag_in_q = dpool.tile([QROWS, SL], dt)
            ag_out_q = dpool.tile([NC_ * QROWS, SL], dt, addr_space="Shared")
            a2a_in = [dpool.tile([PANEL, HID], dt, name=f"a2a_in{p}")
                      for p in range(NPANEL - 1)]
            a2a_out = [dpool.tile([PANEL, HID], dt, name=f"a2a_out{p}")
                       for p in range(NPANEL - 1)]
            # last panel exchanged in two hid-halves
            a2a_lin = [dpool.tile([PANEL, HID // 2], dt, name=f"a2a_lin{i}") for i in range(2)]
            a2a_lout = [dpool.tile([PANEL, HID // 2], dt, name=f"a2a_lout{i}") for i in range(2)]

            # ---- constants ----
            ocol = rp.tile([128, 1], dt)
            orow = rp.tile([1, 128], f32r)
            orow_bf = rp.tile([1, 128], dt)
            invc_t = rp.tile([ROPE, 1], f32)
            nc.sync.dma_start(out=ocol[:], in_=ones_col[:])
            nc.sync.dma_start(out=orow[:], in_=ones_row[:].bitcast(f32r))
            nc.sync.dma_start(out=invc_t[:], in_=invf_col[:])
            nc.vector.tensor_copy(orow_bf[:], orow[:].bitcast(f32))

            # stage B weight tiles (DMAs chained below)
            mask_sb = rp.tile([128, 4 * PANEL], dt, name="mask_sb")
            wqb_all = rp.tile([128, QCH * HPC * 256], dt, name="wqb_all")
            wkk_all = rp.tile([128, KCH * HPC * NOPE], dt, name="wkk_all")
            wkv_all = rp.tile([128, KCH * HPC * VD], dt, name="wkv_all")
            wo_all = rp.tile([128, HPC * HID], dt, name="wo_all")

            def wqb_t(l):
                return wqb_all[:, 512 * l:512 * (l + 1)]

            def wkk_t(l):
                return wkk_all[:, 256 * l:256 * (l + 1)]

            def wkv_t(l):
                return wkv_all[:, 256 * l:256 * (l + 1)]

            def wo_sb(h):
                return wo_all[:, HID * h:HID * (h + 1)]

            sin_all = None
            cos_all = None

            # ================= Stage A =================
            with tc.tile_pool(name="sa_in", bufs=1) as sap, \
                 tc.tile_pool(name="sa_tmp", bufs=2) as tp, \
                 tc.tile_pool(name="sa_ps", bufs=6, space="PSUM") as accp, \
                 tc.tile_pool(name="sa_ps1", bufs=1, space="PSUM") as pp1:

                # x and wa_kv in parallel; everything later is chained behind
                # them with 1-element anchor copies on the (otherwise idle)
                # gpsimd queue so one HWDGE queue never interleaves a later
                # transfer with an earlier, urgent one.
                x_all = sap.tile([128, HCH * SL], dt, name="x_all")
                nc.sync.dma_start(out=x_all[:], in_=x_p[:])
                wakv_all = sap.tile([128, HCH * KVW], dt, name="wakv_all")
                nc.sync.dma_start(out=wakv_all[:], in_=wakv_p[:])
                waq_all = sap.tile([128, HCH * QLR], dt, name="waq_all")
                NQQ = 4
                wq = HCH * QLR // NQQ
                prev_anchor = wakv_all
                chain = []
                for qq in range(NQQ):
                    dst = waq_all[:, wq * qq:wq * (qq + 1)]
                    nc.gpsimd.tensor_copy(waq_all[0:1, wq * qq:wq * qq + 1],
                                          prev_anchor[0:1, 0:1])
                    nc.sync.dma_start(out=dst, in_=waq_p[:, wq * qq:wq * (qq + 1)])
                    prev_anchor = waq_all[:, wq * qq:wq * (qq + 1)]
                for wtile, wsrc in ((wkk_all, wkk_p), (wkv_all, wkv_p), (wqb_all, wqb_p),
                                    (mask_sb, mask_in), (wo_all, wo_p)):
                    nc.gpsimd.tensor_copy(wtile[0:1, 0:1], prev_anchor[0:1, 0:1])
                    nc.sync.dma_start(out=wtile[:], in_=wsrc[:])
                    prev_anchor = wtile

                def xt(hc):
                    return x_all[:, SL * hc:SL * (hc + 1)]

                pos_all_t = tp.tile([1, S], f32r, name="pos_all_t", tag="posa", bufs=1)
                pos_t = tp.tile([1, SL], f32r, name="pos_t", tag="poso", bufs=1)
                nc.sync.dma_start(out=pos_all_t[:], in_=pos_all[:].bitcast(f32r))
                nc.sync.dma_start(out=pos_t[:], in_=pos[:].bitcast(f32r))
                emb_all = tp.tile([ROPE, S], f32, name="emb_all", tag="emba", bufs=1)

                # rope angle tables via K=1 outer products (one PSUM bank per
                # accumulation group -- matmul start zeroes a whole bank)
                for j in range(S // SL):
                    tb = accp.tile([128, SL], f32, name=f"tb_all{j}", tag="acc", bufs=6)
                    nc.tensor.matmul(tb[0:ROPE, :], orow[0:1, 0:ROPE],
                                     pos_all_t[:, SL * j:SL * (j + 1)], start=True, stop=True)
                    nc.vector.tensor_scalar(out=emb_all[:, SL * j:SL * (j + 1)],
                                            in0=tb[0:ROPE, :], scalar1=invc_t[:],
                                            scalar2=None, op0=ALU.mult)
                tb_own = accp.tile([128, SL], f32, name="tb_own", tag="acc", bufs=6)
                nc.tensor.matmul(tb_own[0:ROPE, 0:SL], orow[0:1, 0:ROPE], pos_t[:],
                                 start=True, stop=True)
                emb_own = tp.tile([ROPE, SL], f32, name="emb_own", tag="emb_own", bufs=1)
                nc.vector.tensor_scalar(out=emb_own[:], in0=tb_own[0:ROPE, 0:SL],
                                        scalar1=invc_t[:], scalar2=None, op0=ALU.mult)

                sin_all = _range_reduce_sin(nc, tp, emb_all[:], ROPE, S, 0.0, "sa",
                                            res_pool=rp, res_dt=dt, tagw="w")
                cos_all = _range_reduce_sin(nc, tp, emb_all[:], ROPE, S, PI / 2, "ca",
                                            res_pool=rp, res_dt=dt, tagw="w")
                sin_own = _range_reduce_sin(nc, tp, emb_own[:], ROPE, SL, 0.0, "so")
                cos_own = _range_reduce_sin(nc, tp, emb_own[:], ROPE, SL, PI / 2, "co")

                # ---- kv wave: chunks c0..c3 + pe accumulate over all hc ----
                acc_kv = [accp.tile([128, SL], f32, name=f"acc_kv{c}", tag="acc", bufs=6)
                          for c in range(KCH)]
                acc_pe = accp.tile([128, SL], f32, name="acc_pe", tag="acc", bufs=6)
                for hc in range(HCH):
                    st = (hc == 0)
                    sp = (hc == HCH - 1)
                    for c in range(KCH):
                        nc.tensor.matmul(acc_kv[c][:],
                                         wakv_all[:, KVW * hc + 128 * c:KVW * hc + 128 * (c + 1)],
                                         xt(hc), start=st, stop=sp)
                    nc.tensor.matmul(acc_pe[0:ROPE, :],
                                     wakv_all[:, KVW * hc + KVLR:KVW * hc + KVW],
                                     xt(hc), start=st, stop=sp)

                # kv ssq + rms scale
                ssq_kv = pp1.tile([1, SL], f32, name="ssq_kv", tag="ssq", bufs=1)
                sqs = []
                for c in range(KCH):
                    sq = tp.tile([128, SL], dt, name=f"sqk{c}", tag="sq", bufs=4)
                    nc.scalar.activation(sq[:], acc_kv[c][:], AF.Square)
                    sqs.append(sq)
                for c in range(KCH):
                    nc.tensor.matmul(ssq_kv[:], ocol[:], sqs[c][:],
                                     start=(c == 0), stop=(c == KCH - 1))
                ms_kv = tp.tile([1, SL], f32, name="ms_kv", tag="ms", bufs=2)
                nc.scalar.activation(ms_kv[:], ssq_kv[:], AF.Sqrt, scale=1.0 / KVLR)
                rkv = tp.tile([1, SL], f32, name="rkv", tag="rr", bufs=2)
                nc.vector.reciprocal_approx_fast(out=rkv[:], in_=ms_kv[:])
                rkvr = tp.tile([1, SL], f32r, name="rkvr", tag="rrr", bufs=2)
                with nc.allow_low_precision(reason="f32r rounding of rms scale"):
                    nc.vector.tensor_copy(rkvr[:], rkv[:])
                bc_kv = pp1.tile([128, SL], f32, name="bc_kv", tag="bc", bufs=1)
                nc.tensor.matmul(bc_kv[:], orow[:], rkvr[:], start=True, stop=True)
                bckv_sb = tp.tile([128, SL], f32, name="bckv_sb", tag="bc_sb", bufs=2)
                nc.scalar.activation(bckv_sb[:], bc_kv[:], AF.Copy)

                # k_pe rope
                krot = tp.tile([ROPE, SL], f32, name="krot", tag="krot", bufs=1)
                nc.vector.tensor_scalar(out=krot[0:32, :], in0=acc_pe[32:64, :],
                                        scalar1=-1.0, scalar2=None, op0=ALU.mult)
                nc.vector.tensor_copy(krot[32:64, :], acc_pe[0:32, :])
                kro = tp.tile([ROPE, SL], f32, name="kro", tag="kro", bufs=1)
                nc.vector.tensor_mul(kro[:], acc_pe[0:ROPE, :], cos_own[:])
                krs = tp.tile([ROPE, SL], f32, name="krs", tag="krs", bufs=1)
                nc.vector.tensor_mul(krs[:], krot[:], sin_own[:])
                kfin = tp.tile([ROPE, SL], dt, name="kfin", tag="kfin", bufs=1)
                nc.vector.tensor_add(kfin[:], kro[:], krs[:])
                nc.scalar.dma_start(out=ag_in_kv[KVLR:KVLR + ROPE, :], in_=kfin[:])

                for c in range(KCH):
                    sc = tp.tile([128, SL], dt, name=f"sck{c}", tag="sc", bufs=4)
                    nc.vector.tensor_mul(sc[:], acc_kv[c][:], bckv_sb[:])
                    nc.scalar.dma_start(out=ag_in_kv[128 * c:128 * (c + 1), :], in_=sc[:])

                nc.gpsimd.collective_compute(
                    "AllGather", ALU.bypass,
                    replica_groups=[list(range(NC_))],
                    ins=[ag_in_kv[:]], outs=[ag_out_kv[:]],
                )

                # ---- q wave in two PSUM halves of 6 chunks, gathered RAW ----
                ssq_q = pp1.tile([1, SL], f32, name="ssq_q", tag="ssq", bufs=1)
                sqq = []
                acc_q1 = [accp.tile([128, SL], f32, name=f"acc_q1_{c}", tag="acc", bufs=6)
                          for c in range(QHALF)]
                for hc in range(HCH):
                    st = (hc == 0)
                    sp = (hc == HCH - 1)
                    for c in range(QHALF):
                        nc.tensor.matmul(acc_q1[c][:],
                                         waq_all[:, QLR * hc + 128 * c:QLR * hc + 128 * (c + 1)],
                                         xt(hc), start=st, stop=sp)
                for c in range(QHALF):
                    raw = tp.tile([128, SL], dt, name=f"rawqa{c}", tag="sc", bufs=4)
                    nc.vector.tensor_copy(raw[:], acc_q1[c][:])
                    nc.scalar.dma_start(out=ag_in_q[128 * c:128 * (c + 1), :], in_=raw[:])
                    sq = tp.tile([128, SL], dt, name=f"sqq{c}", tag="sq", bufs=4)
                    nc.scalar.activation(sq[:], acc_q1[c][:], AF.Square)
                    sqq.append(sq)
                for c in range(QHALF):
                    nc.tensor.matmul(ssq_q[:], ocol[:], sqq[c][:],
                                     start=(c == 0), stop=False)

                acc_q2 = [accp.tile([128, SL], f32, name=f"acc_q2_{c}", tag="acc", bufs=6)
                          for c in range(QHALF)]
                for hc in range(HCH):
                    st = (hc == 0)
                    sp = (hc == HCH - 1)
                    for c in range(QHALF):
                        cc = c + QHALF
                        nc.tensor.matmul(acc_q2[c][:],
                                         waq_all[:, QLR * hc + 128 * cc:QLR * hc + 128 * (cc + 1)],
                                         xt(hc), start=st, stop=sp)
                for c in range(QHALF):
                    raw = tp.tile([128, SL], dt, name=f"rawqb{c}", tag="sc", bufs=4)
                    nc.vector.tensor_copy(raw[:], acc_q2[c][:])
                    nc.scalar.dma_start(out=ag_in_q[128 * (c + QHALF):128 * (c + QHALF + 1), :],
                                        in_=raw[:])
                    sq = tp.tile([128, SL], dt, name=f"sqq{c + QHALF}", tag="sq", bufs=4)
                    nc.scalar.activation(sq[:], acc_q2[c][:], AF.Square)
                    sqq.append(sq)
                for c in range(QHALF):
                    nc.tensor.matmul(ssq_q[:], ocol[:], sqq[c + QHALF][:],
                                     start=False, stop=(c == QHALF - 1))
                ms_q = tp.tile([1, SL], f32, name="ms_q", tag="ms", bufs=2)
                nc.scalar.activation(ms_q[:], ssq_q[:], AF.Sqrt, scale=1.0 / QLR)
                rq = tp.tile([1, SL], f32, name="rq", tag="rr", bufs=2)
                nc.vector.reciprocal_approx_fast(out=rq[:], in_=ms_q[:])
                r_bf = tp.tile([1, SL], dt, name="r_bf", tag="rbf", bufs=1)
                nc.vector.tensor_copy(r_bf[:], rq[:])
                nc.scalar.dma_start(out=ag_in_q[QCH * 128:QCH * 128 + 1, :], in_=r_bf[:])
                nc.gpsimd.collective_compute(
                    "AllGather", ALU.bypass,
                    replica_groups=[list(range(NC_))],
                    ins=[ag_in_q[:]], outs=[ag_out_q[:]],
                )

            agkv_r = ag_out_kv.rearrange("(r c) q -> r c q", r=NC_)
            agq_r = ag_out_q.rearrange("(r c) q -> r c q", r=NC_)

            # ================= Stage B =================
            with tc.tile_pool(name="sb_res", bufs=1) as sbp, \
                 tc.tile_pool(name="sb_tmp", bufs=2) as tp, \
                 tc.tile_pool(name="sb_qa", bufs=2) as qap, \
                 tc.tile_pool(name="sb_pt", bufs=4) as ptp, \
                 tc.tile_pool(name="sb_mm", bufs=2, space="PSUM") as pmm, \
                 tc.tile_pool(name="sb_at", bufs=2, space="PSUM") as pat, \
                 tc.tile_pool(name="sb_ps1", bufs=1, space="PSUM") as pp1, \
                 tc.tile_pool(name="sb_wo", bufs=2, space="PSUM") as pwo:

                # K^T and V (both heads)
                kpe_g = sbp.tile([ROPE, S], dt, name="kpe_g")
                for r in range(NC_):
                    nc.sync.dma_start(out=kpe_g[:, SL * r:SL * (r + 1)],
                                      in_=agkv_r[r, KVLR:KVLR + ROPE, :])
                kT = [sbp.tile([128, S], dt, name=f"kT{h}") for h in range(HPC)]
                v_t = [sbp.tile([128, HPC * VD], dt, name=f"v_t{kb}") for kb in range(NKB)]
                with tc.tile_pool(name="sb_ckv", bufs=1) as ckvp:
                    ckv_g = []
                    for j in range(KCH):
                        t = ckvp.tile([128, S], dt, name=f"ckv_g{j}")
                        for r in range(NC_):
                            nc.sync.dma_start(out=t[:, SL * r:SL * (r + 1)],
                                              in_=agkv_r[r, 128 * j:128 * (j + 1), :])
                        ckv_g.append(t)
                    ei = 0
                    for h in range(HPC):
                        for kc in range(S // 512):
                            ps = pmm.tile([128, 512], f32, name=f"kt_ps{h}_{kc}", tag="mm", bufs=2)
                            for l in range(KCH):
                                nc.tensor.matmul(ps[:], wkk_t(l)[:, NOPE * h:NOPE * (h + 1)],
                                                 ckv_g[l][:, 512 * kc:512 * (kc + 1)],
                                                 start=(l == 0), stop=(l == KCH - 1))
                            if ei % 2 == 0:
                                nc.vector.tensor_copy(kT[h][:, 512 * kc:512 * (kc + 1)], ps[:])
                            else:
                                nc.scalar.activation(kT[h][:, 512 * kc:512 * (kc + 1)], ps[:], AF.Copy)
                            ei += 1
                    for kb in range(NKB):
                        ps = pmm.tile([128, HPC * VD], f32, name=f"v_ps{kb}", tag="mm", bufs=2)
                        for l in range(KCH):
                            nc.tensor.matmul(ps[:], ckv_g[l][:, 128 * kb:128 * (kb + 1)],
                                             wkv_t(l), start=(l == 0), stop=(l == KCH - 1))
                        if kb % 2 == 0:
                            nc.vector.tensor_copy(v_t[kb][:], ps[:])
                        else:
                            nc.scalar.activation(v_t[kb][:], ps[:], AF.Copy)

                # ---- q projections (single pass, rms scale applied here) ----
                qa_p = {}
                for p in range(NPANEL):
                    for l in range(QCH):
                        t = qap.tile([128, PANEL], dt, name=f"qa_p{p}_{l}", tag="qaa", bufs=16)
                        for r in range(2):
                            nc.sync.dma_start(out=t[:, SL * r:SL * (r + 1)],
                                              in_=agq_r[2 * p + r, 128 * l:128 * (l + 1), :])
                        qa_p[(p, l)] = t
                qn_sb = {}
                qp_sb = {}
                for p in range(NPANEL):
                    qs = slice(PANEL * p, PANEL * (p + 1))
                    rrow = tp.tile([1, PANEL], dt, name=f"rrow{p}", tag="rrow", bufs=2)
                    for r in range(2):
                        nc.sync.dma_start(out=rrow[0:1, SL * r:SL * (r + 1)],
                                          in_=agq_r[2 * p + r, QCH * 128:QCH * 128 + 1, :])
                    rbc = pp1.tile([128, PANEL], f32, name=f"rbc{p}", tag="bcb", bufs=1)
                    nc.tensor.matmul(rbc[:], orow_bf[:], rrow[:], start=True, stop=True)
                    rbc_sb = tp.tile([128, PANEL], f32, name=f"rbc_sb{p}", tag="rbc_sb", bufs=2)
                    nc.scalar.activation(rbc_sb[:], rbc[:], AF.Copy)
                    for h in range(HPC):
                        hcol = 256 * h
                        ps_qn = pmm.tile([128, PANEL], f32, name=f"qn_ps{h}_{p}", tag="mm", bufs=2)
                        for l in range(QCH):
                            nc.tensor.matmul(ps_qn[:], wqb_t(l)[:, hcol:hcol + NOPE],
                                             qa_p[(p, l)][:], start=(l == 0), stop=(l == QCH - 1))
                        ps_qr = pmm.tile([128, PANEL], f32, name=f"qr_ps{h}_{p}", tag="mm", bufs=2)
                        for l in range(QCH):
                            nc.tensor.matmul(ps_qr[:], wqb_t(l)[:, hcol + NOPE:hcol + 256],
                                             qa_p[(p, l)][:], start=(l == 0), stop=(l == QCH - 1))
                        qn = sbp.tile([128, PANEL], dt, name=f"qn_sb{h}_{p}")
                        nc.vector.tensor_mul(qn[:], ps_qn[:], rbc_sb[:])
                        qn_sb[(h, p)] = qn
                        qt1 = tp.tile([ROPE, PANEL], f32, name=f"qt1_{h}_{p}", tag="qt1", bufs=2)
                        nc.vector.tensor_mul(qt1[:], ps_qr[0:ROPE, :], cos_all[:, qs])
                        qt2 = tp.tile([ROPE, PANEL], f32, name=f"qt2_{h}_{p}", tag="qt2", bufs=2)
                        nc.vector.tensor_mul(qt2[:], ps_qr[ROPE:2 * ROPE, :], sin_all[:, qs])
                        qpp = tp.tile([ROPE, PANEL], f32, name=f"qpp{h}_{p}", tag="qpp", bufs=2)
                        nc.vector.tensor_add(qpp[:], qt1[:], qt2[:])
                        qp = sbp.tile([ROPE, PANEL], dt, name=f"qp_sb{h}_{p}")
                        nc.vector.tensor_mul(qp[:], qpp[:], rbc_sb[0:ROPE, :])
                        qp_sb[(h, p)] = qp

                # ---- attention + per-panel Wo partials + AllToAll + reduce ----
                with tc.tile_pool(name="sb_red", bufs=1) as redp:
                    def reduce_rows(srcs, dst_rows, tagsuf):
                        """f32 chain-reduce NC_ bf16 blocks, store to out_loc."""
                        acc = None
                        for r in range(1, NC_):
                            nxt = redp.tile(srcs[r].shape, f32, name=f"racc{tagsuf}_{r}",
                                            tag=f"racc{srcs[r].shape[1]}", bufs=2)
                            if acc is None:
                                nc.vector.tensor_add(nxt[:], srcs[0][:], srcs[1][:])
                            else:
                                nc.vector.tensor_add(nxt[:], acc[:], srcs[r][:])
                            acc = nxt
                        nc.scalar.dma_start(out=dst_rows, in_=acc[:])

                    def reduce_panel(p):
                        rbs = []
                        for r in range(NC_):
                            t = redp.tile([SHARD, HID], dt, name=f"rb{p}_{r}", tag="rb", bufs=4)
                            nc.sync.dma_start(out=t[:], in_=a2a_out[p][SHARD * r:SHARD * (r + 1), :])
                            rbs.append(t)
                        reduce_rows(rbs, out_loc[SHARD * p:SHARD * (p + 1), :], f"p{p}")

                    for p in range(NPANEL):
                        at_ps = {}
                        for h in range(HPC):
                            nkb = 4 * (p + 1)
                            ps_at = pat.tile([128, PANEL], f32, name=f"at_ps{h}_{p}", tag="at", bufs=2)
                            ps_sum = pp1.tile([1, PANEL], f32, name=f"sum_ps{h}_{p}", tag="sum", bufs=1)
                            pts = {}

                            def consume(kb, nkb=nkb, ps_at=ps_at, ps_sum=ps_sum, pts=pts, h=h):
                                nc.tensor.matmul(ps_at[:], v_t[kb][:, VD * h:VD * (h + 1)], pts[kb][:],
                                                 start=(kb == 0), stop=(kb == nkb - 1))
                                nc.tensor.matmul(ps_sum[:], ocol[:], pts[kb][:],
                                                 start=(kb == 0), stop=(kb == nkb - 1))

                            for kb in range(nkb):
                                ps_sc = pmm.tile([128, PANEL], f32, name=f"sc_ps{h}_{p}_{kb}",
                                                 tag="mm", bufs=2)
                                nc.tensor.matmul(ps_sc[:], kT[h][:, 128 * kb:128 * (kb + 1)],
                                                 qn_sb[(h, p)][:], start=True, stop=False)
                                nc.tensor.matmul(ps_sc[:], kpe_g[:, 128 * kb:128 * (kb + 1)],
                                                 qp_sb[(h, p)][:], start=False, stop=True)
                                pt = ptp.tile([128, PANEL], dt, name=f"pt{h}_{p}_{kb}", tag="pt", bufs=4)
                                if kb >= 4 * p:
                                    j = kb - 4 * p
                                    msc = tp.tile([128, PANEL], f32, name=f"msc{h}_{p}_{kb}",
                                                  tag="msc", bufs=2)
                                    nc.vector.tensor_add(msc[:], ps_sc[:],
                                                         mask_sb[:, PANEL * j:PANEL * (j + 1)])
                                    nc.scalar.activation(pt[:], msc[:], AF.Exp)
                                else:
                                    nc.scalar.activation(pt[:], ps_sc[:], AF.Exp)
                                pts[kb] = pt
                                if kb > 0:
                                    consume(kb - 1)
                            consume(nkb - 1)
                            rec = tp.tile([1, PANEL], f32, name=f"rec{h}_{p}", tag="rec", bufs=2)
                            nc.vector.reciprocal_approx_fast(out=rec[:], in_=ps_sum[:])
                            recr = tp.tile([1, PANEL], f32r, name=f"recr{h}_{p}", tag="recr", bufs=2)
                            with nc.allow_low_precision(reason="f32r rounding of softmax recip"):
                                nc.vector.tensor_copy(recr[:], rec[:])
                            bc = pp1.tile([128, PANEL], f32, name=f"bc_ps{h}_{p}", tag="bcb", bufs=1)
                            nc.tensor.matmul(bc[:], orow[:], recr[:], start=True, stop=True)
                            bc_sb = tp.tile([128, PANEL], f32, name=f"bc_sb{h}_{p}", tag="bc_sb", bufs=2)
                            nc.scalar.activation(bc_sb[:], bc[:], AF.Copy)
                            at_p = tp.tile([128, PANEL], dt, name=f"at_p{h}_{p}", tag="at_p", bufs=3)
                            nc.vector.tensor_mul(at_p[:], ps_at[:], bc_sb[:])
                            at_ps[h] = at_p

                        if p > 0 and p < NPANEL - 1:
                            reduce_panel(p - 1)

                        if p < NPANEL - 1:
                            # Wo partial, full hid width, one store per seq block
                            for sb in range(4):
                                ev = tp.tile([128, HID], dt, name=f"woev{p}_{sb}", tag="woev", bufs=2)
                                for n in range(4):
                                    ps_o = pwo.tile([128, 512], f32, name=f"wo_ps{p}_{sb}_{n}",
                                                    tag="wo", bufs=2)
                                    for h in range(HPC):
                                        nc.tensor.matmul(ps_o[:], at_ps[h][:, 128 * sb:128 * (sb + 1)],
                                                         wo_sb(h)[:, 512 * n:512 * (n + 1)],
                                                         start=(h == 0), stop=(h == HPC - 1))
                                    if sb % 2 == 0:
                                        nc.vector.tensor_copy(ev[:, 512 * n:512 * (n + 1)], ps_o[:])
                                    else:
                                        nc.scalar.activation(ev[:, 512 * n:512 * (n + 1)], ps_o[:],
                                                             AF.Copy)
                                nc.scalar.dma_start(out=a2a_in[p][128 * sb:128 * (sb + 1), :],
                                                    in_=ev[:])
                            nc.gpsimd.collective_compute(
                                "AllToAll", ALU.bypass,
                                replica_groups=[list(range(NC_))],
                                ins=[a2a_in[p][:]], outs=[a2a_out[p][:]],
                            )
                        else:
                            # last panel: exchange in two hid halves so the first
                            # half's reduce overlaps the second half's transfer
                            reduce_panel(p - 1)
                            for half in range(2):
                                hid0 = (HID // 2) * half
                                for sb in range(4):
                                    ev = tp.tile([128, HID // 2], dt, name=f"wol{half}_{sb}",
                                                 tag="wolev", bufs=2)
                                    for n in range(2):
                                        ps_o = pwo.tile([128, 512], f32, name=f"wol_ps{half}_{sb}_{n}",
                                                        tag="wo", bufs=2)
                                        for h in range(HPC):
                                            nc.tensor.matmul(
                                                ps_o[:], at_ps[h][:, 128 * sb:128 * (sb + 1)],
                                                wo_sb(h)[:, hid0 + 512 * n:hid0 + 512 * (n + 1)],
                                                start=(h == 0), stop=(h == HPC - 1))
                                        if sb % 2 == 0:
                                            nc.vector.tensor_copy(ev[:, 512 * n:512 * (n + 1)], ps_o[:])
                                        else:
                                            nc.scalar.activation(ev[:, 512 * n:512 * (n + 1)],
                                                                 ps_o[:], AF.Copy)
                                    nc.scalar.dma_start(out=a2a_lin[half][128 * sb:128 * (sb + 1), :],
                                                        in_=ev[:])
                                nc.gpsimd.collective_compute(
                                    "AllToAll", ALU.bypass,
                                    replica_groups=[list(range(NC_))],
                                    ins=[a2a_lin[half][:]], outs=[a2a_lout[half][:]],
                                )
                            for half in range(2):
                                hid0 = (HID // 2) * half
                                rbs = []
                                for r in range(NC_):
                                    t = redp.tile([SHARD, HID // 2], dt, name=f"rbl{half}_{r}",
                                                  tag="rbl", bufs=4)
                                    nc.sync.dma_start(out=t[:],
                                                      in_=a2a_lout[half][SHARD * r:SHARD * (r + 1), :])
                                    rbs.append(t)
                                reduce_rows(rbs,
                                            out_loc[SHARD * p:SHARD * (p + 1), hid0:hid0 + HID // 2],
                                            f"l{half}")

    nc.compile()
    return nc


def _to_dt(a, dt):
    if dt == bf16:
        return np.ascontiguousarray(a.astype(ml_dtypes.bfloat16))
    return np.ascontiguousarray(a.astype(np.float32))


def _prepare_inputs(dt, hidden_states, position_ids, Wqa, qa_ln_w, Wqb, Wkva, kv_ln_w, Wkvb, Wo):
    perm = np.concatenate([np.arange(0, ROPE, 2), np.arange(1, ROPE, 2)])
    X = np.asarray(hidden_states, np.float32).reshape(S, HID)
    pos_f = np.ascontiguousarray(np.asarray(position_ids, np.float32).reshape(1, S))
    Wqa = np.asarray(Wqa, np.float32)
    Wkva = np.asarray(Wkva, np.float32)
    wa_kv = np.concatenate([Wkva[:, :KVLR], Wkva[:, KVLR:][:, perm]], axis=1)  # (2048, 576)
    wqb_base = np.asarray(Wqb, np.float32) * np.asarray(qa_ln_w, np.float32)[:, None]
    wkvb_base = np.asarray(Wkvb, np.float32) * np.asarray(kv_ln_w, np.float32)[:, None]
    Wo = np.asarray(Wo, np.float32)

    head_blocks = []
    for h in range(NH):
        cols = wqb_base[:, 192 * h:192 * (h + 1)] * SM_SCALE
        nope = cols[:, :NOPE]
        pe_d = cols[:, NOPE:][:, perm]
        rot = np.concatenate([-pe_d[:, 32:], pe_d[:, :32]], axis=1)
        head_blocks.append(np.concatenate([nope, pe_d, rot], axis=1))  # (1536, 256)
    k_blocks = [wkvb_base[:, 256 * h:256 * h + NOPE] for h in range(NH)]
    v_blocks = [wkvb_base[:, 256 * h + NOPE:256 * (h + 1)] for h in range(NH)]

    inv = (1.0 / (THETA ** (np.arange(0, ROPE, 2, dtype=np.float32) / ROPE))).astype(np.float32)
    invf_np = np.concatenate([inv, inv])

    # diagonal masks: block j, mask[r, col] = NEG where col < 128 j + r
    colsi = np.arange(PANEL)[None, :]
    rowsi = np.arange(128)[:, None]
    mask_np = np.concatenate(
        [np.where(colsi < 128 * j + rowsi, NEG, 0.0) for j in range(4)], axis=1
    ).astype(np.float32)

    def pack_rows(w):
        ch = w.shape[0] // 128
        return np.concatenate([w[128 * k:128 * (k + 1), :] for k in range(ch)], axis=1)

    wa_kv_d = _to_dt(pack_rows(wa_kv), dt)
    wa_q_d = _to_dt(pack_rows(Wqa), dt)
    mask_d = _to_dt(mask_np, dt)
    ones_col_d = _to_dt(np.ones((128, 1), np.float32), dt)

    in_maps = []
    for c in range(NC_):
        rows_c = slice(SL * c, SL * (c + 1))
        wqb_c = np.concatenate([head_blocks[HPC * c + h] for h in range(HPC)], axis=1)
        wkk_c = np.concatenate([k_blocks[HPC * c + h] for h in range(HPC)], axis=1)
        wkv_c = np.concatenate([v_blocks[HPC * c + h] for h in range(HPC)], axis=1)
        wo_c = np.concatenate([Wo[VD * (HPC * c + h):VD * (HPC * c + h + 1), :]
                               for h in range(HPC)], axis=0)
        in_maps.append({
            "x_p": _to_dt(pack_rows(np.ascontiguousarray(X[rows_c, :].T)), dt),
            "pos": np.ascontiguousarray(pos_f[:, rows_c]),
            "pos_all": pos_f,
            "wakv_p": wa_kv_d,
            "waq_p": wa_q_d,
            "wqb_p": _to_dt(pack_rows(wqb_c), dt),
            "wkk_p": _to_dt(pack_rows(wkk_c), dt),
            "wkv_p": _to_dt(pack_rows(wkv_c), dt),
            "wo_p": _to_dt(pack_rows(wo_c), dt),
            "mask": mask_d,
            "ones_col": ones_col_d,
            "ones_row": np.ones((1, 128), np.float32),
            "invf_col": invf_np.reshape(ROPE, 1).copy(),
        })
    return in_maps


def run(inputs, trace=False, trace_cores=None, dt=None):
    dt = dt if dt is not None else DT
    key = ("nc", str(dt))
    if key not in _CACHE:
        _CACHE[key] = build_program(dt)
    nc = _CACHE[key]
    in_maps = _prepare_inputs(dt, **inputs)
    res = run_bass_kernel_spmd(nc, in_maps, list(range(NC_)), trace=trace,
                               trace_cores=trace_cores)
    # reassemble: panel p, core c holds global seq rows [512 p + 64 c, 512 p + 64 (c+1))
    out = np.empty((S, HID), np.float32)
    for c in range(NC_):
        o = res.results[c]["out_loc"]
        for p in range(NPANEL):
            out[PANEL * p + SHARD * c:PANEL * p + SHARD * (c + 1), :] = \
                o[SHARD * p:SHARD * (p + 1), :]
    return out.reshape(1, S, HID), res


def kernel(**inputs) -> np.ndarray:
    out, _ = run(inputs, trace=False)
    return out


# revision 27
# speedup vs baseline: 1.0692x; 1.0022x over previous
"""DeepseekV3 MLA flash-attention prefill kernel for 8 Trainium2 NeuronCores.

Sharding (SPMD, one program for all 8 cores):
  Stage A (sequence-parallel): core c owns 256 seq rows. Inputs arrive as a
    dependency-chained sequence of large packed DMAs (x || wa_kv, then the
    wa_q quarters, then stage-B weights) so early tiles are never delayed by
    later transfers interleaving on the same queue. Each weight wave
    accumulates into bank-exclusive PSUM groups. The kv AllGather fires right
    at the initial-barrier horizon; the q AllGather carries RAW (unnormalized)
    qa plus the rms scale row, applied post-projection in stage B.
  Stage B (head-parallel): core c owns heads {2c, 2c+1}. K^T/V from the kv
    gather. Causal attention in (k, q) layout, no max-subtraction,
    fully-masked k-blocks skipped, diagonal blocks masked by a vector
    mask-add (softmax scale pre-folded into Wqb host-side).
  Output: per-panel partial Wo products (only this core's 2 head-rows of Wo)
    are exchanged with one AllToAll per 512-row panel and reduced on-core in
    f32; earlier panels' exchanges hide under later (heavier) panels'
    attention. The last panel's exchange is split into two hid-halves so its
    first half's reduction overlaps the second half's transfer.
"""

import sys

if '/opt/trn_rl_repo' not in sys.path:
    sys.path.insert(0, '/opt/trn_rl_repo')

import numpy as np
import ml_dtypes

import concourse.bass as bass
import concourse.mybir as mybir
import concourse.tile as tile
from concourse import bacc
from concourse.bass_utils import run_bass_kernel_spmd

f32 = mybir.dt.float32
f32r = mybir.dt.float32r
bf16 = mybir.dt.bfloat16
i32 = mybir.dt.int32
AF = mybir.ActivationFunctionType
ALU = mybir.AluOpType

NC_ = 8            # cores
S = 2048           # sequence
HID = 2048
QLR = 1536         # q lora rank
KVLR = 512         # kv lora rank
ROPE = 64
NOPE = 128
VD = 128
NH = 16
HPC = NH // NC_    # heads per core = 2
SL = S // NC_      # rows per core = 256
PANEL = 512        # q panel width
NPANEL = S // PANEL
NKB = S // 128     # 16 k blocks
QCH = QLR // 128   # 12
QHALF = QCH // 2   # 6
KCH = KVLR // 128  # 4
HCH = HID // 128   # 16
KVW = KVLR + ROPE  # 576 = kv wave width
SHARD = PANEL // NC_  # 64 rows per core per panel
THETA = 10000.0
SM_SCALE = float((NOPE + ROPE) ** -0.5)
PI = float(np.pi)
NEG = -1e30

DT = bf16

_CACHE = {}


def _range_reduce_sin(nc, pool, src_ap, P, W, bias, name, res_pool=None, res_dt=f32, tagw=""):
    """sin(src + bias) with range reduction to [-pi, pi]. src may be PSUM."""
    t0 = pool.tile([P, W], f32, name=f"{name}_t0", tag=f"rr0{tagw}", bufs=1)
    ti = pool.tile([P, W], i32, name=f"{name}_ti", tag=f"rr1{tagw}", bufs=1)
    tf = pool.tile([P, W], f32, name=f"{name}_tf", tag=f"rr2{tagw}", bufs=1)
    arg = pool.tile([P, W], f32, name=f"{name}_arg", tag=f"rr3{tagw}", bufs=1)
    res = (res_pool or pool).tile([P, W], res_dt, name=f"{name}_sin", tag=f"res_{name}", bufs=1)
    nc.vector.tensor_scalar(out=t0[:], in0=src_ap, scalar1=bias, scalar2=None, op0=ALU.add)
    nc.vector.tensor_scalar(out=tf[:], in0=t0[:], scalar1=1.0 / (2 * PI), scalar2=None, op0=ALU.mult)
    nc.vector.tensor_copy(ti[:], tf[:])
    nc.vector.tensor_copy(tf[:], ti[:])
    nc.vector.scalar_tensor_tensor(out=arg[:], in0=tf[:], scalar=-2 * PI, in1=t0[:], op0=ALU.mult, op1=ALU.add)
    nc.scalar.activation(res[:], arg[:], AF.Sin)
    return res


def build_program(dt):
    nc = bacc.Bacc("TRN2", target_bir_lowering=False, debug=False, num_devices=NC_)

    def din(name, shape):
        return nc.dram_tensor(name, shape, dt, kind="ExternalInput")

    # ---- external I/O (per-core data, packed for large-row DMAs) ----
    x_p = din("x_p", [128, HCH * SL])            # hc-major packed X^T
    pos = nc.dram_tensor("pos", [1, SL], f32, kind="ExternalInput")
    pos_all = nc.dram_tensor("pos_all", [1, S], f32, kind="ExternalInput")
    wakv_p = din("wakv_p", [128, HCH * KVW])     # [Wkva(kv)|Wkva(pe,deint)] per hc
    waq_p = din("waq_p", [128, HCH * QLR])       # Wqa per hc
    wqb_p = din("wqb_p", [128, QCH * HPC * 256])  # [nope|pe_d|rot]*SM per head, per l
    wkk_p = din("wkk_p", [128, KCH * HPC * NOPE])
    wkv_p = din("wkv_p", [128, KCH * HPC * VD])
    wo_p = din("wo_p", [128, HPC * HID])         # Wo rows for this core's heads
    mask_in = din("mask", [128, 4 * PANEL])      # diag masks j=0..3 (0 / -1e30)
    ones_col = din("ones_col", [128, 1])
    ones_row = nc.dram_tensor("ones_row", [1, 128], f32, kind="ExternalInput")
    invf_col = nc.dram_tensor("invf_col", [ROPE, 1], f32, kind="ExternalInput")
    out_loc = nc.dram_tensor("out_loc", [NPANEL * SHARD, HID], f32, kind="ExternalOutput")

    QROWS = QCH * 128 + 1  # 12 raw chunks + rms scale row

    with tile.TileContext(nc) as tc:
        with tc.tile_pool(name="dram", bufs=1, space="DRAM") as dpool, \
             tc.tile_pool(name="persist", bufs=1) as rp:
            ag_in_kv = dpool.tile([KVW, SL], dt)
            ag_out_kv = dpool.tile([NC_ * KVW, SL], dt, addr_space="Shared")
            ag_in_q = dpool.tile([QROWS, SL], dt)
            ag_out_q = dpool.tile([NC_ * QROWS, SL], dt, addr_space="Shared")
            a2a_in = {p: dpool.tile([PANEL, HID], dt, name=f"a2a_in{p}")
                      for p in range(1, NPANEL)}
            a2a_out = {p: dpool.tile([PANEL, HID], dt, name=f"a2a_out{p}")
                       for p in range(1, NPANEL)}
            # last panel exchanged in two hid-halves
            a2a_lin = [dpool.tile([PANEL, HID // 2], dt, name=f"a2a_lin{i}") for i in range(2)]
            a2a_lout = [dpool.tile([PANEL, HID // 2], dt, name=f"a2a_lout{i}") for i in range(2)]

            # ---- constants ----
            ocol = rp.tile([128, 1], dt)
            orow = rp.tile([1, 128], f32r)
            orow_bf = rp.tile([1, 128], dt)
            invc_t = rp.tile([ROPE, 1], f32)
            nc.sync.dma_start(out=ocol[:], in_=ones_col[:])
            nc.sync.dma_start(out=orow[:], in_=ones_row[:].bitcast(f32r))
            nc.sync.dma_start(out=invc_t[:], in_=invf_col[:])
            nc.vector.tensor_copy(orow_bf[:], orow[:].bitcast(f32))

            # stage B weight tiles (DMAs chained below)
            mask_sb = rp.tile([128, 4 * PANEL], dt, name="mask_sb")
            wqb_all = rp.tile([128, QCH * HPC * 256], dt, name="wqb_all")
            wkk_all = rp.tile([128, KCH * HPC * NOPE], dt, name="wkk_all")
            wkv_all = rp.tile([128, KCH * HPC * VD], dt, name="wkv_all")
            wo_all = rp.tile([128, HPC * HID], dt, name="wo_all")

            def wqb_t(l):
                return wqb_all[:, 512 * l:512 * (l + 1)]

            def wkk_t(l):
                return wkk_all[:, 256 * l:256 * (l + 1)]

            def wkv_t(l):
                return wkv_all[:, 256 * l:256 * (l + 1)]

            def wo_sb(h):
                return wo_all[:, HID * h:HID * (h + 1)]

            sin_all = None
            cos_all = None

            # ================= Stage A =================
            with tc.tile_pool(name="sa_in", bufs=1) as sap, \
                 tc.tile_pool(name="sa_tmp", bufs=2) as tp, \
                 tc.tile_pool(name="sa_ps", bufs=6, space="PSUM") as accp, \
                 tc.tile_pool(name="sa_ps1", bufs=1, space="PSUM") as pp1:

                # x and wa_kv in parallel; everything later is chained behind
                # them with 1-element anchor copies on the (otherwise idle)
                # gpsimd queue so one HWDGE queue never interleaves a later
                # transfer with an earlier, urgent one.
                x_all = sap.tile([128, HCH * SL], dt, name="x_all")
                nc.sync.dma_start(out=x_all[:], in_=x_p[:])
                wakv_all = sap.tile([128, HCH * KVW], dt, name="wakv_all")
                nc.sync.dma_start(out=wakv_all[:], in_=wakv_p[:])
                waq_all = sap.tile([128, HCH * QLR], dt, name="waq_all")
                NQQ = 4
                wq = HCH * QLR // NQQ
                prev_anchor = wakv_all
                for qq in range(NQQ):
                    dst = waq_all[:, wq * qq:wq * (qq + 1)]
                    nc.gpsimd.tensor_copy(waq_all[0:1, wq * qq:wq * qq + 1],
                                          prev_anchor[0:1, 0:1])
                    nc.sync.dma_start(out=dst, in_=waq_p[:, wq * qq:wq * (qq + 1)])
                    prev_anchor = waq_all[:, wq * qq:wq * (qq + 1)]

                def chain_weight_dmas():
                    prev = waq_all[:, wq * (NQQ - 1):wq * NQQ]
                    for wtile, wsrc in ((wkk_all, wkk_p), (wkv_all, wkv_p), (wqb_all, wqb_p),
                                        (mask_sb, mask_in), (wo_all, wo_p)):
                        nc.gpsimd.tensor_copy(wtile[0:1, 0:1], prev[0:1, 0:1])
                        nc.sync.dma_start(out=wtile[:], in_=wsrc[:])
                        prev = wtile

                def xt(hc):
                    return x_all[:, SL * hc:SL * (hc + 1)]

                pos_all_t = tp.tile([1, S], f32r, name="pos_all_t", tag="posa", bufs=1)
                pos_t = tp.tile([1, SL], f32r, name="pos_t", tag="poso", bufs=1)
                nc.sync.dma_start(out=pos_all_t[:], in_=pos_all[:].bitcast(f32r))
                nc.sync.dma_start(out=pos_t[:], in_=pos[:].bitcast(f32r))
                emb_all = tp.tile([ROPE, S], f32, name="emb_all", tag="emba", bufs=1)

                # rope angle tables via K=1 outer products (one PSUM bank per
                # accumulation group -- matmul start zeroes a whole bank)
                for j in range(S // SL):
                    tb = accp.tile([128, SL], f32, name=f"tb_all{j}", tag="acc", bufs=6)
                    nc.tensor.matmul(tb[0:ROPE, :], orow[0:1, 0:ROPE],
                                     pos_all_t[:, SL * j:SL * (j + 1)], start=True, stop=True)
                    nc.vector.tensor_scalar(out=emb_all[:, SL * j:SL * (j + 1)],
                                            in0=tb[0:ROPE, :], scalar1=invc_t[:],
                                            scalar2=None, op0=ALU.mult)
                tb_own = accp.tile([128, SL], f32, name="tb_own", tag="acc", bufs=6)
                nc.tensor.matmul(tb_own[0:ROPE, 0:SL], orow[0:1, 0:ROPE], pos_t[:],
                                 start=True, stop=True)
                emb_own = tp.tile([ROPE, SL], f32, name="emb_own", tag="emb_own", bufs=1)
                nc.vector.tensor_scalar(out=emb_own[:], in0=tb_own[0:ROPE, 0:SL],
                                        scalar1=invc_t[:], scalar2=None, op0=ALU.mult)

                sin_all = _range_reduce_sin(nc, tp, emb_all[:], ROPE, S, 0.0, "sa",
                                            res_pool=rp, res_dt=dt, tagw="w")
                cos_all = _range_reduce_sin(nc, tp, emb_all[:], ROPE, S, PI / 2, "ca",
                                            res_pool=rp, res_dt=dt, tagw="w")
                sin_own = _range_reduce_sin(nc, tp, emb_own[:], ROPE, SL, 0.0, "so")
                cos_own = _range_reduce_sin(nc, tp, emb_own[:], ROPE, SL, PI / 2, "co")

                # ---- kv wave: chunks c0..c3 + pe accumulate over all hc ----
                acc_kv = [accp.tile([128, SL], f32, name=f"acc_kv{c}", tag="acc", bufs=6)
                          for c in range(KCH)]
                acc_pe = accp.tile([128, SL], f32, name="acc_pe", tag="acc", bufs=6)
                for hc in range(HCH):
                    st = (hc == 0)
                    sp = (hc == HCH - 1)
                    for c in range(KCH):
                        nc.tensor.matmul(acc_kv[c][:],
                                         wakv_all[:, KVW * hc + 128 * c:KVW * hc + 128 * (c + 1)],
                                         xt(hc), start=st, stop=sp)
                    nc.tensor.matmul(acc_pe[0:ROPE, :],
                                     wakv_all[:, KVW * hc + KVLR:KVW * hc + KVW],
                                     xt(hc), start=st, stop=sp)

                # kv ssq + rms scale
                ssq_kv = pp1.tile([1, SL], f32, name="ssq_kv", tag="ssq", bufs=1)
                sqs = []
                for c in range(KCH):
                    sq = tp.tile([128, SL], dt, name=f"sqk{c}", tag="sq", bufs=4)
                    nc.scalar.activation(sq[:], acc_kv[c][:], AF.Square)
                    sqs.append(sq)
                for c in range(KCH):
                    nc.tensor.matmul(ssq_kv[:], ocol[:], sqs[c][:],
                                     start=(c == 0), stop=(c == KCH - 1))
                ms_kv = tp.tile([1, SL], f32, name="ms_kv", tag="ms", bufs=2)
                nc.scalar.activation(ms_kv[:], ssq_kv[:], AF.Sqrt, scale=1.0 / KVLR)
                rkv = tp.tile([1, SL], f32, name="rkv", tag="rr", bufs=2)
                nc.vector.reciprocal_approx_fast(out=rkv[:], in_=ms_kv[:])
                rkvr = tp.tile([1, SL], f32r, name="rkvr", tag="rrr", bufs=2)
                with nc.allow_low_precision(reason="f32r rounding of rms scale"):
                    nc.vector.tensor_copy(rkvr[:], rkv[:])
                bc_kv = pp1.tile([128, SL], f32, name="bc_kv", tag="bc", bufs=1)
                nc.tensor.matmul(bc_kv[:], orow[:], rkvr[:], start=True, stop=True)
                bckv_sb = tp.tile([128, SL], f32, name="bckv_sb", tag="bc_sb", bufs=2)
                nc.scalar.activation(bckv_sb[:], bc_kv[:], AF.Copy)

                # k_pe rope
                krot = tp.tile([ROPE, SL], f32, name="krot", tag="krot", bufs=1)
                nc.vector.tensor_scalar(out=krot[0:32, :], in0=acc_pe[32:64, :],
                                        scalar1=-1.0, scalar2=None, op0=ALU.mult)
                nc.vector.tensor_copy(krot[32:64, :], acc_pe[0:32, :])
                kro = tp.tile([ROPE, SL], f32, name="kro", tag="kro", bufs=1)
                nc.vector.tensor_mul(kro[:], acc_pe[0:ROPE, :], cos_own[:])
                krs = tp.tile([ROPE, SL], f32, name="krs", tag="krs", bufs=1)
                nc.vector.tensor_mul(krs[:], krot[:], sin_own[:])
                kfin = tp.tile([ROPE, SL], dt, name="kfin", tag="kfin", bufs=1)
                nc.vector.tensor_add(kfin[:], kro[:], krs[:])
                nc.scalar.dma_start(out=ag_in_kv[KVLR:KVLR + ROPE, :], in_=kfin[:])

                for c in range(KCH):
                    sc = tp.tile([128, SL], dt, name=f"sck{c}", tag="sc", bufs=4)
                    nc.vector.tensor_mul(sc[:], acc_kv[c][:], bckv_sb[:])
                    nc.scalar.dma_start(out=ag_in_kv[128 * c:128 * (c + 1), :], in_=sc[:])

                nc.gpsimd.collective_compute(
                    "AllGather", ALU.bypass,
                    replica_groups=[list(range(NC_))],
                    ins=[ag_in_kv[:]], outs=[ag_out_kv[:]],
                )
                chain_weight_dmas()

                # ---- q wave in two PSUM halves of 6 chunks, gathered RAW ----
                ssq_q = pp1.tile([1, SL], f32, name="ssq_q", tag="ssq", bufs=1)
                sqq = []
                acc_q1 = [accp.tile([128, SL], f32, name=f"acc_q1_{c}", tag="acc", bufs=6)
                          for c in range(QHALF)]
                for hc in range(HCH):
                    st = (hc == 0)
                    sp = (hc == HCH - 1)
                    for c in range(QHALF):
                        nc.tensor.matmul(acc_q1[c][:],
                                         waq_all[:, QLR * hc + 128 * c:QLR * hc + 128 * (c + 1)],
                                         xt(hc), start=st, stop=sp)
                for c in range(QHALF):
                    raw = tp.tile([128, SL], dt, name=f"rawqa{c}", tag="sc", bufs=4)
                    nc.vector.tensor_copy(raw[:], acc_q1[c][:])
                    nc.scalar.dma_start(out=ag_in_q[128 * c:128 * (c + 1), :], in_=raw[:])
                    sq = tp.tile([128, SL], dt, name=f"sqq{c}", tag="sq", bufs=4)
                    nc.scalar.activation(sq[:], acc_q1[c][:], AF.Square)
                    sqq.append(sq)
                for c in range(QHALF):
                    nc.tensor.matmul(ssq_q[:], ocol[:], sqq[c][:],
                                     start=(c == 0), stop=False)

                acc_q2 = [accp.tile([128, SL], f32, name=f"acc_q2_{c}", tag="acc", bufs=6)
                          for c in range(QHALF)]
                for hc in range(HCH):
                    st = (hc == 0)
                    sp = (hc == HCH - 1)
                    for c in range(QHALF):
                        cc = c + QHALF
                        nc.tensor.matmul(acc_q2[c][:],
                                         waq_all[:, QLR * hc + 128 * cc:QLR * hc + 128 * (cc + 1)],
                                         xt(hc), start=st, stop=sp)
                for c in range(QHALF):
                    raw = tp.tile([128, SL], dt, name=f"rawqb{c}", tag="sc", bufs=4)
                    nc.vector.tensor_copy(raw[:], acc_q2[c][:])
                    nc.scalar.dma_start(out=ag_in_q[128 * (c + QHALF):128 * (c + QHALF + 1), :],
                                        in_=raw[:])
                    sq = tp.tile([128, SL], dt, name=f"sqq{c + QHALF}", tag="sq", bufs=4)
                    nc.scalar.activation(sq[:], acc_q2[c][:], AF.Square)
                    sqq.append(sq)
                for c in range(QHALF):
                    nc.tensor.matmul(ssq_q[:], ocol[:], sqq[c + QHALF][:],
                                     start=False, stop=(c == QHALF - 1))
                ms_q = tp.tile([1, SL], f32, name="ms_q", tag="ms", bufs=2)
                nc.scalar.activation(ms_q[:], ssq_q[:], AF.Sqrt, scale=1.0 / QLR)
                rq = tp.tile([1, SL], f32, name="rq", tag="rr", bufs=2)
                nc.vector.reciprocal_approx_fast(out=rq[:], in_=ms_q[:])
                r_bf = tp.tile([1, SL], dt, name="r_bf", tag="rbf", bufs=1)
                nc.vector.tensor_copy(r_bf[:], rq[:])
                nc.scalar.dma_start(out=ag_in_q[QCH * 128:QCH * 128 + 1, :], in_=r_bf[:])
                nc.gpsimd.collective_compute(
                    "AllGather", ALU.bypass,
                    replica_groups=[list(range(NC_))],
                    ins=[ag_in_q[:]], outs=[ag_out_q[:]],
                )

            agkv_r = ag_out_kv.rearrange("(r c) q -> r c q", r=NC_)
            agq_r = ag_out_q.rearrange("(r c) q -> r c q", r=NC_)

            # ================= Stage B =================
            with tc.tile_pool(name="sb_res", bufs=1) as sbp, \
                 tc.tile_pool(name="sb_tmp", bufs=2) as tp, \
                 tc.tile_pool(name="sb_qa", bufs=2) as qap, \
                 tc.tile_pool(name="sb_pt", bufs=4) as ptp, \
                 tc.tile_pool(name="sb_mm", bufs=2, space="PSUM") as pmm, \
                 tc.tile_pool(name="sb_at", bufs=2, space="PSUM") as pat, \
                 tc.tile_pool(name="sb_ps1", bufs=1, space="PSUM") as pp1, \
                 tc.tile_pool(name="sb_wo", bufs=2, space="PSUM") as pwo:

                # K^T and V (both heads)
                kpe_g = sbp.tile([ROPE, S], dt, name="kpe_g")
                for r in range(NC_):
                    nc.sync.dma_start(out=kpe_g[:, SL * r:SL * (r + 1)],
                                      in_=agkv_r[r, KVLR:KVLR + ROPE, :])
                kT = [sbp.tile([128, S], dt, name=f"kT{h}") for h in range(HPC)]
                v_t = [sbp.tile([128, HPC * VD], dt, name=f"v_t{kb}") for kb in range(NKB)]
                with tc.tile_pool(name="sb_ckv", bufs=1) as ckvp:
                    ckv_g = []
                    for j in range(KCH):
                        t = ckvp.tile([128, S], dt, name=f"ckv_g{j}")
                        for r in range(NC_):
                            nc.sync.dma_start(out=t[:, SL * r:SL * (r + 1)],
                                              in_=agkv_r[r, 128 * j:128 * (j + 1), :])
                        ckv_g.append(t)
                    ei = 0
                    for h in range(HPC):
                        for kc in range(S // 512):
                            ps = pmm.tile([128, 512], f32, name=f"kt_ps{h}_{kc}", tag="mm", bufs=2)
                            for l in range(KCH):
                                nc.tensor.matmul(ps[:], wkk_t(l)[:, NOPE * h:NOPE * (h + 1)],
                                                 ckv_g[l][:, 512 * kc:512 * (kc + 1)],
                                                 start=(l == 0), stop=(l == KCH - 1))
                            if ei % 2 == 0:
                                nc.vector.tensor_copy(kT[h][:, 512 * kc:512 * (kc + 1)], ps[:])
                            else:
                                nc.scalar.activation(kT[h][:, 512 * kc:512 * (kc + 1)], ps[:], AF.Copy)
                            ei += 1
                    for kb in range(NKB):
                        ps = pmm.tile([128, HPC * VD], f32, name=f"v_ps{kb}", tag="mm", bufs=2)
                        for l in range(KCH):
                            nc.tensor.matmul(ps[:], ckv_g[l][:, 128 * kb:128 * (kb + 1)],
                                             wkv_t(l), start=(l == 0), stop=(l == KCH - 1))
                        if kb % 2 == 0:
                            nc.vector.tensor_copy(v_t[kb][:], ps[:])
                        else:
                            nc.scalar.activation(v_t[kb][:], ps[:], AF.Copy)

                # ---- q projections (single pass, rms scale applied here) ----
                qa_p = {}
                for p in range(NPANEL - 1, -1, -1):   # processing order
                    for l in range(QCH):
                        t = qap.tile([128, PANEL], dt, name=f"qa_p{p}_{l}", tag="qaa", bufs=16)
                        for r in range(2):
                            nc.sync.dma_start(out=t[:, SL * r:SL * (r + 1)],
                                              in_=agq_r[2 * p + r, 128 * l:128 * (l + 1), :])
                        qa_p[(p, l)] = t
                qn_sb = {}
                qp_sb = {}

                def qproj_panel(p):
                    qs = slice(PANEL * p, PANEL * (p + 1))
                    rrow = tp.tile([1, PANEL], dt, name=f"rrow{p}", tag="rrow", bufs=2)
                    for r in range(2):
                        nc.sync.dma_start(out=rrow[0:1, SL * r:SL * (r + 1)],
                                          in_=agq_r[2 * p + r, QCH * 128:QCH * 128 + 1, :])
                    rbc = pp1.tile([128, PANEL], f32, name=f"rbc{p}", tag="bcb", bufs=1)
                    nc.tensor.matmul(rbc[:], orow_bf[:], rrow[:], start=True, stop=True)
                    rbc_sb = tp.tile([128, PANEL], f32, name=f"rbc_sb{p}", tag="rbc_sb", bufs=2)
                    nc.scalar.activation(rbc_sb[:], rbc[:], AF.Copy)
                    for h in range(HPC):
                        hcol = 256 * h
                        ps_qn = pmm.tile([128, PANEL], f32, name=f"qn_ps{h}_{p}", tag="mm", bufs=2)
                        for l in range(QCH):
                            nc.tensor.matmul(ps_qn[:], wqb_t(l)[:, hcol:hcol + NOPE],
                                             qa_p[(p, l)][:], start=(l == 0), stop=(l == QCH - 1))
                        ps_qr = pmm.tile([128, PANEL], f32, name=f"qr_ps{h}_{p}", tag="mm", bufs=2)
                        for l in range(QCH):
                            nc.tensor.matmul(ps_qr[:], wqb_t(l)[:, hcol + NOPE:hcol + 256],
                                             qa_p[(p, l)][:], start=(l == 0), stop=(l == QCH - 1))
                        qn = sbp.tile([128, PANEL], dt, name=f"qn_sb{h}_{p}")
                        nc.vector.tensor_mul(qn[:], ps_qn[:], rbc_sb[:])
                        qn_sb[(h, p)] = qn
                        qt1 = tp.tile([ROPE, PANEL], f32, name=f"qt1_{h}_{p}", tag="qt1", bufs=2)
                        nc.vector.tensor_mul(qt1[:], ps_qr[0:ROPE, :], cos_all[:, qs])
                        qt2 = tp.tile([ROPE, PANEL], f32, name=f"qt2_{h}_{p}", tag="qt2", bufs=2)
                        nc.vector.tensor_mul(qt2[:], ps_qr[ROPE:2 * ROPE, :], sin_all[:, qs])
                        qpp = tp.tile([ROPE, PANEL], f32, name=f"qpp{h}_{p}", tag="qpp", bufs=2)
                        nc.vector.tensor_add(qpp[:], qt1[:], qt2[:])
                        qp = sbp.tile([ROPE, PANEL], dt, name=f"qp_sb{h}_{p}")
                        nc.vector.tensor_mul(qp[:], qpp[:], rbc_sb[0:ROPE, :])
                        qp_sb[(h, p)] = qp

                # ---- attention + per-panel Wo partials + AllToAll + reduce ----
                # heaviest panel first so its exchange hides under later panels
                PLIST = list(range(NPANEL - 1, -1, -1))
                with tc.tile_pool(name="sb_red", bufs=1) as redp:
                    def reduce_rows(srcs, dst_rows, tagsuf):
                        """f32 chain-reduce NC_ bf16 blocks, store to out_loc."""
                        acc = None
                        for r in range(1, NC_):
                            nxt = redp.tile(srcs[r].shape, f32, name=f"racc{tagsuf}_{r}",
                                            tag=f"racc{srcs[r].shape[1]}", bufs=2)
                            if acc is None:
                                nc.vector.tensor_add(nxt[:], srcs[0][:], srcs[1][:])
                            else:
                                nc.vector.tensor_add(nxt[:], acc[:], srcs[r][:])
                            acc = nxt
                        nc.sync.dma_start(out=dst_rows, in_=acc[:])

                    def reduce_panel(p):
                        rbs = []
                        for r in range(NC_):
                            t = redp.tile([SHARD, HID], dt, name=f"rb{p}_{r}", tag="rb", bufs=4)
                            nc.sync.dma_start(out=t[:], in_=a2a_out[p][SHARD * r:SHARD * (r + 1), :])
                            rbs.append(t)
                        reduce_rows(rbs, out_loc[SHARD * p:SHARD * (p + 1), :], f"p{p}")

                    for i, p in enumerate(PLIST):
                        last = (i == len(PLIST) - 1)
                        qproj_panel(p)
                        at_ps = {}
                        for h in range(HPC):
                            nkb = 4 * (p + 1)
                            ps_at = pat.tile([128, PANEL], f32, name=f"at_ps{h}_{p}", tag="at", bufs=2)
                            ps_sum = pp1.tile([1, PANEL], f32, name=f"sum_ps{h}_{p}", tag="sum", bufs=1)
                            pts = {}

                            def consume(kb, nkb=nkb, ps_at=ps_at, ps_sum=ps_sum, pts=pts, h=h):
                                nc.tensor.matmul(ps_at[:], v_t[kb][:, VD * h:VD * (h + 1)], pts[kb][:],
                                                 start=(kb == 0), stop=(kb == nkb - 1))
                                nc.tensor.matmul(ps_sum[:], ocol[:], pts[kb][:],
                                                 start=(kb == 0), stop=(kb == nkb - 1))

                            for kb in range(nkb):
                                ps_sc = pmm.tile([128, PANEL], f32, name=f"sc_ps{h}_{p}_{kb}",
                                                 tag="mm", bufs=2)
                                nc.tensor.matmul(ps_sc[:], kT[h][:, 128 * kb:128 * (kb + 1)],
                                                 qn_sb[(h, p)][:], start=True, stop=False)
                                nc.tensor.matmul(ps_sc[:], kpe_g[:, 128 * kb:128 * (kb + 1)],
                                                 qp_sb[(h, p)][:], start=False, stop=True)
                                pt = ptp.tile([128, PANEL], dt, name=f"pt{h}_{p}_{kb}", tag="pt", bufs=4)
                                if kb >= 4 * p:
                                    j = kb - 4 * p
                                    msc = tp.tile([128, PANEL], f32, name=f"msc{h}_{p}_{kb}",
                                                  tag="msc", bufs=2)
                                    nc.vector.tensor_add(msc[:], ps_sc[:],
                                                         mask_sb[:, PANEL * j:PANEL * (j + 1)])
                                    nc.scalar.activation(pt[:], msc[:], AF.Exp)
                                else:
                                    nc.scalar.activation(pt[:], ps_sc[:], AF.Exp)
                                pts[kb] = pt
                                if kb > 0:
                                    consume(kb - 1)
                            consume(nkb - 1)
                            rec = tp.tile([1, PANEL], f32, name=f"rec{h}_{p}", tag="rec", bufs=2)
                            nc.vector.reciprocal_approx_fast(out=rec[:], in_=ps_sum[:])
                            recr = tp.tile([1, PANEL], f32r, name=f"recr{h}_{p}", tag="recr", bufs=2)
                            with nc.allow_low_precision(reason="f32r rounding of softmax recip"):
                                nc.vector.tensor_copy(recr[:], rec[:])
                            bc = pp1.tile([128, PANEL], f32, name=f"bc_ps{h}_{p}", tag="bcb", bufs=1)
                            nc.tensor.matmul(bc[:], orow[:], recr[:], start=True, stop=True)
                            bc_sb = tp.tile([128, PANEL], f32, name=f"bc_sb{h}_{p}", tag="bc_sb", bufs=2)
                            nc.scalar.activation(bc_sb[:], bc[:], AF.Copy)
                            at_p = tp.tile([128, PANEL], dt, name=f"at_p{h}_{p}", tag="at_p", bufs=3)
                            nc.vector.tensor_mul(at_p[:], ps_at[:], bc_sb[:])
                            at_ps[h] = at_p

                        if i > 0:
                            reduce_panel(PLIST[i - 1])

                        if not last:
                            # Wo partial, full hid width, one store per seq block
                            for sb in range(4):
                                ev = tp.tile([128, HID], dt, name=f"woev{p}_{sb}", tag="woev", bufs=2)
                                for n in range(4):
                                    ps_o = pwo.tile([128, 512], f32, name=f"wo_ps{p}_{sb}_{n}",
                                                    tag="wo", bufs=2)
                                    for h in range(HPC):
                                        nc.tensor.matmul(ps_o[:], at_ps[h][:, 128 * sb:128 * (sb + 1)],
                                                         wo_sb(h)[:, 512 * n:512 * (n + 1)],
                                                         start=(h == 0), stop=(h == HPC - 1))
                                    if sb % 2 == 0:
                                        nc.vector.tensor_copy(ev[:, 512 * n:512 * (n + 1)], ps_o[:])
                                    else:
                                        nc.scalar.activation(ev[:, 512 * n:512 * (n + 1)], ps_o[:],
                                                             AF.Copy)
                                nc.scalar.dma_start(out=a2a_in[p][128 * sb:128 * (sb + 1), :],
                                                    in_=ev[:])
                            nc.gpsimd.collective_compute(
                                "AllToAll", ALU.bypass,
                                replica_groups=[list(range(NC_))],
                                ins=[a2a_in[p][:]], outs=[a2a_out[p][:]],
                            )
                        else:
                            # last processed (lightest) panel: exchange in two hid
                            # halves so the first half's reduce overlaps the second
                            for half in range(2):
                                hid0 = (HID // 2) * half
                                for sb in range(4):
                                    ev = tp.tile([128, HID // 2], dt, name=f"wol{half}_{sb}",
                                                 tag="wolev", bufs=2)
                                    for n in range(2):
                                        ps_o = pwo.tile([128, 512], f32, name=f"wol_ps{half}_{sb}_{n}",
                                                        tag="wo", bufs=2)
                                        for h in range(HPC):
                                            nc.tensor.matmul(
                                                ps_o[:], at_ps[h][:, 128 * sb:128 * (sb + 1)],
                                                wo_sb(h)[:, hid0 + 512 * n:hid0 + 512 * (n + 1)],
                                                start=(h == 0), stop=(h == HPC - 1))
                                        if sb % 2 == 0:
                                            nc.vector.tensor_copy(ev[:, 512 * n:512 * (n + 1)], ps_o[:])
                                        else:
                                            nc.scalar.activation(ev[:, 512 * n:512 * (n + 1)],
                                                                 ps_o[:], AF.Copy)
                                    nc.scalar.dma_start(out=a2a_lin[half][128 * sb:128 * (sb + 1), :],
                                                        in_=ev[:])
                                nc.gpsimd.collective_compute(
                                    "AllToAll", ALU.bypass,
                                    replica_groups=[list(range(NC_))],
                                    ins=[a2a_lin[half][:]], outs=[a2a_lout[half][:]],
                                )
                            for half in range(2):
                                hid0 = (HID // 2) * half
                                rbs = []
                                for r in range(NC_):
                                    t = redp.tile([SHARD, HID // 2], dt, name=f"rbl{half}_{r}",
                                                  tag="rbl", bufs=4)
                                    nc.sync.dma_start(out=t[:],
                                                      in_=a2a_lout[half][SHARD * r:SHARD * (r + 1), :])
                                    rbs.append(t)
                                reduce_rows(rbs,
                                            out_loc[SHARD * p:SHARD * (p + 1), hid0:hid0 + HID // 2],
                                            f"l{half}")

    nc.compile()
    return nc


def _to_dt(a, dt):
    if dt == bf16:
        return np.ascontiguousarray(a.astype(ml_dtypes.bfloat16))
    return np.ascontiguousarray(a.astype(np.float32))


def _prepare_inputs(dt, hidden_states, position_ids, Wqa, qa_ln_w, Wqb, Wkva, kv_ln_w, Wkvb, Wo):
    perm = np.concatenate([np.arange(0, ROPE, 2), np.arange(1, ROPE, 2)])
    X = np.asarray(hidden_states, np.float32).reshape(S, HID)
    pos_f = np.ascontiguousarray(np.asarray(position_ids, np.float32).reshape(1, S))
    Wqa = np.asarray(Wqa, np.float32)
    Wkva = np.asarray(Wkva, np.float32)
    wa_kv = np.concatenate([Wkva[:, :KVLR], Wkva[:, KVLR:][:, perm]], axis=1)  # (2048, 576)
    wqb_base = np.asarray(Wqb, np.float32) * np.asarray(qa_ln_w, np.float32)[:, None]
    wkvb_base = np.asarray(Wkvb, np.float32) * np.asarray(kv_ln_w, np.float32)[:, None]
    Wo = np.asarray(Wo, np.float32)

    head_blocks = []
    for h in range(NH):
        cols = wqb_base[:, 192 * h:192 * (h + 1)] * SM_SCALE
        nope = cols[:, :NOPE]
        pe_d = cols[:, NOPE:][:, perm]
        rot = np.concatenate([-pe_d[:, 32:], pe_d[:, :32]], axis=1)
        head_blocks.append(np.concatenate([nope, pe_d, rot], axis=1))  # (1536, 256)
    k_blocks = [wkvb_base[:, 256 * h:256 * h + NOPE] for h in range(NH)]
    v_blocks = [wkvb_base[:, 256 * h + NOPE:256 * (h + 1)] for h in range(NH)]

    inv = (1.0 / (THETA ** (np.arange(0, ROPE, 2, dtype=np.float32) / ROPE))).astype(np.float32)
    invf_np = np.concatenate([inv, inv])

    # diagonal masks: block j, mask[r, col] = NEG where col < 128 j + r
    colsi = np.arange(PANEL)[None, :]
    rowsi = np.arange(128)[:, None]
    mask_np = np.concatenate(
        [np.where(colsi < 128 * j + rowsi, NEG, 0.0) for j in range(4)], axis=1
    ).astype(np.float32)

    def pack_rows(w):
        ch = w.shape[0] // 128
        return np.concatenate([w[128 * k:128 * (k + 1), :] for k in range(ch)], axis=1)

    wa_kv_d = _to_dt(pack_rows(wa_kv), dt)
    wa_q_d = _to_dt(pack_rows(Wqa), dt)
    mask_d = _to_dt(mask_np, dt)
    ones_col_d = _to_dt(np.ones((128, 1), np.float32), dt)

    in_maps = []
    for c in range(NC_):
        rows_c = slice(SL * c, SL * (c + 1))
        wqb_c = np.concatenate([head_blocks[HPC * c + h] for h in range(HPC)], axis=1)
        wkk_c = np.concatenate([k_blocks[HPC * c + h] for h in range(HPC)], axis=1)
        wkv_c = np.concatenate([v_blocks[HPC * c + h] for h in range(HPC)], axis=1)
        wo_c = np.concatenate([Wo[VD * (HPC * c + h):VD * (HPC * c + h + 1), :]
                               for h in range(HPC)], axis=0)
        in_maps.append({
            "x_p": _to_dt(pack_rows(np.ascontiguousarray(X[rows_c, :].T)), dt),
            "pos": np.ascontiguousarray(pos_f[:, rows_c]),
            "pos_all": pos_f,
            "wakv_p": wa_kv_d,
            "waq_p": wa_q_d,
            "wqb_p": _to_dt(pack_rows(wqb_c), dt),
            "wkk_p": _to_dt(pack_rows(wkk_c), dt),
            "wkv_p": _to_dt(pack_rows(wkv_c), dt),
            "wo_p": _to_dt(pack_rows(wo_c), dt),
            "mask": mask_d,
            "ones_col": ones_col_d,
            "ones_row": np.ones((1, 128), np.float32),
            "invf_col": invf_np.reshape(ROPE, 1).copy(),
        })
    return in_maps


def run(inputs, trace=False, trace_cores=None, dt=None):
    dt = dt if dt is not None else DT
    key = ("nc", str(dt))
    if key not in _CACHE:
        _CACHE[key] = build_program(dt)
    nc = _CACHE[key]
    in_maps = _prepare_inputs(dt, **inputs)
    res = run_bass_kernel_spmd(nc, in_maps, list(range(NC_)), trace=trace,
                               trace_cores=trace_cores)
    # reassemble: panel p, core c holds global seq rows [512 p + 64 c, 512 p + 64 (c+1))
    out = np.empty((S, HID), np.float32)
    for c in range(NC_):
        o = res.results[c]["out_loc"]
        for p in range(NPANEL):
            out[PANEL * p + SHARD * c:PANEL * p + SHARD * (c + 1), :] = \
                o[SHARD * p:SHARD * (p + 1), :]
    return out.reshape(1, S, HID), res


def kernel(**inputs) -> np.ndarray:
    out, _ = run(inputs, trace=False)
    return out


# revision 30
# speedup vs baseline: 1.0740x; 1.0045x over previous
"""DeepseekV3 MLA flash-attention prefill kernel for 8 Trainium2 NeuronCores.

Sharding (SPMD, one program for all 8 cores):
  Stage A (sequence-parallel): core c owns 256 seq rows. Inputs arrive as a
    dependency-chained sequence of large packed DMAs (x || wa_kv, then the
    wa_q quarters, then stage-B weights) so early tiles are never delayed by
    later transfers interleaving on the same queue. Each weight wave
    accumulates into bank-exclusive PSUM groups. The kv AllGather fires right
    at the initial-barrier horizon; the q AllGather carries RAW (unnormalized)
    qa plus the rms scale row, applied post-projection in stage B.
  Stage B (head-parallel): core c owns heads {2c, 2c+1}. K^T/V from the kv
    gather. Causal attention in (k, q) layout, no max-subtraction,
    fully-masked k-blocks skipped, diagonal blocks masked by a vector
    mask-add (softmax scale pre-folded into Wqb host-side).
  Output: per-panel partial Wo products (only this core's 2 head-rows of Wo)
    are exchanged with one AllToAll per 512-row panel and reduced on-core in
    f32; earlier panels' exchanges hide under later (heavier) panels'
    attention. The last panel's exchange is split into two hid-halves so its
    first half's reduction overlaps the second half's transfer.
"""

import sys

if '/opt/trn_rl_repo' not in sys.path:
    sys.path.insert(0, '/opt/trn_rl_repo')

import numpy as np
import ml_dtypes

import concourse.bass as bass
import concourse.mybir as mybir
import concourse.tile as tile
from concourse import bacc
from concourse.bass_utils import run_bass_kernel_spmd

f32 = mybir.dt.float32
f32r = mybir.dt.float32r
bf16 = mybir.dt.bfloat16
i32 = mybir.dt.int32
AF = mybir.ActivationFunctionType
ALU = mybir.AluOpType

NC_ = 8            # cores
S = 2048           # sequence
HID = 2048
QLR = 1536         # q lora rank
KVLR = 512         # kv lora rank
ROPE = 64
NOPE = 128
VD = 128
NH = 16
HPC = NH // NC_    # heads per core = 2
SL = S // NC_      # rows per core = 256
PANEL = 512        # q panel width
NPANEL = S // PANEL
NKB = S // 128     # 16 k blocks
QCH = QLR // 128   # 12
QHALF = QCH // 2   # 6
KCH = KVLR // 128  # 4
HCH = HID // 128   # 16
KVW = KVLR + ROPE  # 576 = kv wave width
SHARD = PANEL // NC_  # 64 rows per core per panel
THETA = 10000.0
SM_SCALE = float((NOPE + ROPE) ** -0.5)
PI = float(np.pi)
NEG = -1e30

DT = bf16

_CACHE = {}


def _range_reduce_sin(nc, pool, src_ap, P, W, bias, name, res_pool=None, res_dt=f32, tagw=""):
    """sin(src + bias) with range reduction to [-pi, pi]. src may be PSUM."""
    t0 = pool.tile([P, W], f32, name=f"{name}_t0", tag=f"rr0{tagw}", bufs=1)
    ti = pool.tile([P, W], i32, name=f"{name}_ti", tag=f"rr1{tagw}", bufs=1)
    tf = pool.tile([P, W], f32, name=f"{name}_tf", tag=f"rr2{tagw}", bufs=1)
    arg = pool.tile([P, W], f32, name=f"{name}_arg", tag=f"rr3{tagw}", bufs=1)
    res = (res_pool or pool).tile([P, W], res_dt, name=f"{name}_sin", tag=f"res_{name}", bufs=1)
    nc.vector.tensor_scalar(out=t0[:], in0=src_ap, scalar1=bias, scalar2=None, op0=ALU.add)
    nc.vector.tensor_scalar(out=tf[:], in0=t0[:], scalar1=1.0 / (2 * PI), scalar2=None, op0=ALU.mult)
    nc.vector.tensor_copy(ti[:], tf[:])
    nc.vector.tensor_copy(tf[:], ti[:])
    nc.vector.scalar_tensor_tensor(out=arg[:], in0=tf[:], scalar=-2 * PI, in1=t0[:], op0=ALU.mult, op1=ALU.add)
    nc.scalar.activation(res[:], arg[:], AF.Sin)
    return res


def build_program(dt):
    nc = bacc.Bacc("TRN2", target_bir_lowering=False, debug=False, num_devices=NC_)

    def din(name, shape):
        return nc.dram_tensor(name, shape, dt, kind="ExternalInput")

    # ---- external I/O (per-core data, packed for large-row DMAs) ----
    x_p = din("x_p", [128, HCH * SL])            # hc-major packed X^T
    pos = nc.dram_tensor("pos", [1, SL], f32, kind="ExternalInput")
    pos_all = nc.dram_tensor("pos_all", [1, S], f32, kind="ExternalInput")
    wakv_p = din("wakv_p", [128, HCH * KVW])     # [Wkva(kv)|Wkva(pe,deint)] per hc
    waq_p = din("waq_p", [128, HCH * QLR])       # Wqa per hc
    wqb_p = din("wqb_p", [128, QCH * HPC * 256])  # [nope|pe_d|rot]*SM per head, per l
    wkk_p = din("wkk_p", [128, KCH * HPC * NOPE])
    wkv_p = din("wkv_p", [128, KCH * HPC * VD])
    wo_p = din("wo_p", [128, HPC * HID])         # Wo rows for this core's heads
    mask_in = din("mask", [128, 4 * PANEL])      # diag masks j=0..3 (0 / -1e30)
    ones_col = din("ones_col", [128, 1])
    ones_row = nc.dram_tensor("ones_row", [1, 128], f32, kind="ExternalInput")
    invf_col = nc.dram_tensor("invf_col", [ROPE, 1], f32, kind="ExternalInput")
    out_loc = nc.dram_tensor("out_loc", [NPANEL * SHARD, HID], f32, kind="ExternalOutput")

    QROWS = QCH * 128 + 1  # 12 raw chunks + rms scale row

    with tile.TileContext(nc) as tc:
        with tc.tile_pool(name="dram", bufs=1, space="DRAM") as dpool, \
             tc.tile_pool(name="persist", bufs=1) as rp:
            ag_in_kv = dpool.tile([KVW, SL], dt)
            ag_out_kv = dpool.tile([NC_ * KVW, SL], dt, addr_space="Shared")
            ag_in_q = dpool.tile([QROWS, SL], dt)
            ag_out_q = dpool.tile([NC_ * QROWS, SL], dt, addr_space="Shared")
            a2a_in = {p: dpool.tile([PANEL, HID], dt, name=f"a2a_in{p}")
                      for p in range(1, NPANEL)}
            a2a_out = {p: dpool.tile([PANEL, HID], dt, name=f"a2a_out{p}")
                       for p in range(1, NPANEL)}
            # last panel exchanged in two hid-halves
            a2a_lin = [dpool.tile([PANEL, HID // 2], dt, name=f"a2a_lin{i}") for i in range(2)]
            a2a_lout = [dpool.tile([PANEL, HID // 2], dt, name=f"a2a_lout{i}") for i in range(2)]

            # ---- constants ----
            ocol = rp.tile([128, 1], dt)
            orow = rp.tile([1, 128], f32r)
            orow_bf = rp.tile([1, 128], dt)
            invc_t = rp.tile([ROPE, 1], f32)
            nc.sync.dma_start(out=ocol[:], in_=ones_col[:])
            nc.sync.dma_start(out=orow[:], in_=ones_row[:].bitcast(f32r))
            nc.sync.dma_start(out=invc_t[:], in_=invf_col[:])
            nc.vector.tensor_copy(orow_bf[:], orow[:].bitcast(f32))

            # stage B weight tiles (DMAs chained below)
            mask_sb = rp.tile([128, 4 * PANEL], dt, name="mask_sb")
            wqb_all = rp.tile([128, QCH * HPC * 256], dt, name="wqb_all")
            wkk_all = rp.tile([128, KCH * HPC * NOPE], dt, name="wkk_all")
            wkv_all = rp.tile([128, KCH * HPC * VD], dt, name="wkv_all")
            wo_all = rp.tile([128, HPC * HID], dt, name="wo_all")

            def wqb_t(l):
                return wqb_all[:, 512 * l:512 * (l + 1)]

            def wkk_t(l):
                return wkk_all[:, 256 * l:256 * (l + 1)]

            def wkv_t(l):
                return wkv_all[:, 256 * l:256 * (l + 1)]

            def wo_sb(h):
                return wo_all[:, HID * h:HID * (h + 1)]

            sin_all = None
            cos_all = None

            # ================= Stage A =================
            with tc.tile_pool(name="sa_in", bufs=1) as sap, \
                 tc.tile_pool(name="sa_tmp", bufs=2) as tp, \
                 tc.tile_pool(name="sa_ps", bufs=6, space="PSUM") as accp, \
                 tc.tile_pool(name="sa_ps1", bufs=1, space="PSUM") as pp1:

                # x and wa_kv in parallel; everything later is chained behind
                # them with 1-element anchor copies on the (otherwise idle)
                # gpsimd queue so one HWDGE queue never interleaves a later
                # transfer with an earlier, urgent one.
                x_all = sap.tile([128, HCH * SL], dt, name="x_all")
                nc.sync.dma_start(out=x_all[:], in_=x_p[:])
                wakv_all = sap.tile([128, HCH * KVW], dt, name="wakv_all")
                nc.sync.dma_start(out=wakv_all[:], in_=wakv_p[:])
                waq_all = sap.tile([128, HCH * QLR], dt, name="waq_all")
                NQQ = 4
                wq = HCH * QLR // NQQ
                prev_anchor = wakv_all
                for qq in range(NQQ):
                    dst = waq_all[:, wq * qq:wq * (qq + 1)]
                    nc.gpsimd.tensor_copy(waq_all[0:1, wq * qq:wq * qq + 1],
                                          prev_anchor[0:1, 0:1])
                    nc.sync.dma_start(out=dst, in_=waq_p[:, wq * qq:wq * (qq + 1)])
                    prev_anchor = waq_all[:, wq * qq:wq * (qq + 1)]

                def chain_weight_dmas(anchor_tile):
                    for wtile, wsrc in ((wkk_all, wkk_p), (wkv_all, wkv_p), (wqb_all, wqb_p),
                                        (mask_sb, mask_in), (wo_all, wo_p)):
                        nc.gpsimd.tensor_copy(wtile[0:1, 0:1], anchor_tile[0:1, 0:1])
                        nc.sync.dma_start(out=wtile[:], in_=wsrc[:])

                def xt(hc):
                    return x_all[:, SL * hc:SL * (hc + 1)]

                pos_all_t = tp.tile([1, S], f32r, name="pos_all_t", tag="posa", bufs=1)
                pos_t = tp.tile([1, SL], f32r, name="pos_t", tag="poso", bufs=1)
                nc.sync.dma_start(out=pos_all_t[:], in_=pos_all[:].bitcast(f32r))
                nc.sync.dma_start(out=pos_t[:], in_=pos[:].bitcast(f32r))
                emb_all = tp.tile([ROPE, S], f32, name="emb_all", tag="emba", bufs=1)

                # rope angle tables via K=1 outer products (one PSUM bank per
                # accumulation group -- matmul start zeroes a whole bank)
                for j in range(S // SL):
                    tb = accp.tile([128, SL], f32, name=f"tb_all{j}", tag="acc", bufs=6)
                    nc.tensor.matmul(tb[0:ROPE, :], orow[0:1, 0:ROPE],
                                     pos_all_t[:, SL * j:SL * (j + 1)], start=True, stop=True)
                    nc.vector.tensor_scalar(out=emb_all[:, SL * j:SL * (j + 1)],
                                            in0=tb[0:ROPE, :], scalar1=invc_t[:],
                                            scalar2=None, op0=ALU.mult)
                tb_own = accp.tile([128, SL], f32, name="tb_own", tag="acc", bufs=6)
                nc.tensor.matmul(tb_own[0:ROPE, 0:SL], orow[0:1, 0:ROPE], pos_t[:],
                                 start=True, stop=True)
                emb_own = tp.tile([ROPE, SL], f32, name="emb_own", tag="emb_own", bufs=1)
                nc.vector.tensor_scalar(out=emb_own[:], in0=tb_own[0:ROPE, 0:SL],
                                        scalar1=invc_t[:], scalar2=None, op0=ALU.mult)

                sin_all = _range_reduce_sin(nc, tp, emb_all[:], ROPE, S, 0.0, "sa",
                                            res_pool=rp, res_dt=dt, tagw="w")
                cos_all = _range_reduce_sin(nc, tp, emb_all[:], ROPE, S, PI / 2, "ca",
                                            res_pool=rp, res_dt=dt, tagw="w")
                sin_own = _range_reduce_sin(nc, tp, emb_own[:], ROPE, SL, 0.0, "so")
                cos_own = _range_reduce_sin(nc, tp, emb_own[:], ROPE, SL, PI / 2, "co")

                # ---- kv wave: chunks c0..c3 + pe accumulate over all hc ----
                acc_kv = [accp.tile([128, SL], f32, name=f"acc_kv{c}", tag="acc", bufs=6)
                          for c in range(KCH)]
                acc_pe = accp.tile([128, SL], f32, name="acc_pe", tag="acc", bufs=6)
                for hc in range(HCH):
                    st = (hc == 0)
                    sp = (hc == HCH - 1)
                    for c in range(KCH):
                        nc.tensor.matmul(acc_kv[c][:],
                                         wakv_all[:, KVW * hc + 128 * c:KVW * hc + 128 * (c + 1)],
                                         xt(hc), start=st, stop=sp)
                    nc.tensor.matmul(acc_pe[0:ROPE, :],
                                     wakv_all[:, KVW * hc + KVLR:KVW * hc + KVW],
                                     xt(hc), start=st, stop=sp)

                # kv ssq + rms scale
                ssq_kv = pp1.tile([1, SL], f32, name="ssq_kv", tag="ssq", bufs=1)
                sqs = []
                for c in range(KCH):
                    sq = tp.tile([128, SL], dt, name=f"sqk{c}", tag="sq", bufs=4)
                    nc.scalar.activation(sq[:], acc_kv[c][:], AF.Square)
                    sqs.append(sq)
                for c in range(KCH):
                    nc.tensor.matmul(ssq_kv[:], ocol[:], sqs[c][:],
                                     start=(c == 0), stop=(c == KCH - 1))
                ms_kv = tp.tile([1, SL], f32, name="ms_kv", tag="ms", bufs=2)
                nc.scalar.activation(ms_kv[:], ssq_kv[:], AF.Sqrt, scale=1.0 / KVLR)
                rkv = tp.tile([1, SL], f32, name="rkv", tag="rr", bufs=2)
                nc.vector.reciprocal_approx_fast(out=rkv[:], in_=ms_kv[:])
                rkvr = tp.tile([1, SL], f32r, name="rkvr", tag="rrr", bufs=2)
                with nc.allow_low_precision(reason="f32r rounding of rms scale"):
                    nc.vector.tensor_copy(rkvr[:], rkv[:])
                bc_kv = pp1.tile([128, SL], f32, name="bc_kv", tag="bc", bufs=1)
                nc.tensor.matmul(bc_kv[:], orow[:], rkvr[:], start=True, stop=True)
                bckv_sb = tp.tile([128, SL], f32, name="bckv_sb", tag="bc_sb", bufs=2)
                nc.scalar.activation(bckv_sb[:], bc_kv[:], AF.Copy)

                # k_pe rope
                krot = tp.tile([ROPE, SL], f32, name="krot", tag="krot", bufs=1)
                nc.vector.tensor_scalar(out=krot[0:32, :], in0=acc_pe[32:64, :],
                                        scalar1=-1.0, scalar2=None, op0=ALU.mult)
                nc.vector.tensor_copy(krot[32:64, :], acc_pe[0:32, :])
                kro = tp.tile([ROPE, SL], f32, name="kro", tag="kro", bufs=1)
                nc.vector.tensor_mul(kro[:], acc_pe[0:ROPE, :], cos_own[:])
                krs = tp.tile([ROPE, SL], f32, name="krs", tag="krs", bufs=1)
                nc.vector.tensor_mul(krs[:], krot[:], sin_own[:])
                kfin = tp.tile([ROPE, SL], dt, name="kfin", tag="kfin", bufs=1)
                nc.vector.tensor_add(kfin[:], kro[:], krs[:])
                nc.scalar.dma_start(out=ag_in_kv[KVLR:KVLR + ROPE, :], in_=kfin[:])

                last_sck = None
                for c in range(KCH):
                    sc = tp.tile([128, SL], dt, name=f"sck{c}", tag="sc", bufs=4)
                    nc.vector.tensor_mul(sc[:], acc_kv[c][:], bckv_sb[:])
                    nc.scalar.dma_start(out=ag_in_kv[128 * c:128 * (c + 1), :], in_=sc[:])
                    last_sck = sc

                nc.gpsimd.collective_compute(
                    "AllGather", ALU.bypass,
                    replica_groups=[list(range(NC_))],
                    ins=[ag_in_kv[:]], outs=[ag_out_kv[:]],
                )
                chain_weight_dmas(last_sck)

                # ---- q wave in two PSUM halves of 6 chunks, gathered RAW ----
                ssq_q = pp1.tile([1, SL], f32, name="ssq_q", tag="ssq", bufs=1)
                sqq = []
                acc_q1 = [accp.tile([128, SL], f32, name=f"acc_q1_{c}", tag="acc", bufs=6)
                          for c in range(QHALF)]
                for hc in range(HCH):
                    st = (hc == 0)
                    sp = (hc == HCH - 1)
                    for c in range(QHALF):
                        nc.tensor.matmul(acc_q1[c][:],
                                         waq_all[:, QLR * hc + 128 * c:QLR * hc + 128 * (c + 1)],
                                         xt(hc), start=st, stop=sp)
                for c in range(QHALF):
                    raw = tp.tile([128, SL], dt, name=f"rawqa{c}", tag="sc", bufs=4)
                    nc.vector.tensor_copy(raw[:], acc_q1[c][:])
                    nc.scalar.dma_start(out=ag_in_q[128 * c:128 * (c + 1), :], in_=raw[:])
                    sq = tp.tile([128, SL], dt, name=f"sqq{c}", tag="sq", bufs=4)
                    nc.scalar.activation(sq[:], acc_q1[c][:], AF.Square)
                    sqq.append(sq)
                for c in range(QHALF):
                    nc.tensor.matmul(ssq_q[:], ocol[:], sqq[c][:],
                                     start=(c == 0), stop=False)

                acc_q2 = [accp.tile([128, SL], f32, name=f"acc_q2_{c}", tag="acc", bufs=6)
                          for c in range(QHALF)]
                for hc in range(HCH):
                    st = (hc == 0)
                    sp = (hc == HCH - 1)
                    for c in range(QHALF):
                        cc = c + QHALF
                        nc.tensor.matmul(acc_q2[c][:],
                                         waq_all[:, QLR * hc + 128 * cc:QLR * hc + 128 * (cc + 1)],
                                         xt(hc), start=st, stop=sp)
                for c in range(QHALF):
                    raw = tp.tile([128, SL], dt, name=f"rawqb{c}", tag="sc", bufs=4)
                    nc.vector.tensor_copy(raw[:], acc_q2[c][:])
                    nc.scalar.dma_start(out=ag_in_q[128 * (c + QHALF):128 * (c + QHALF + 1), :],
                                        in_=raw[:])
                    sq = tp.tile([128, SL], dt, name=f"sqq{c + QHALF}", tag="sq", bufs=4)
                    nc.scalar.activation(sq[:], acc_q2[c][:], AF.Square)
                    sqq.append(sq)
                for c in range(QHALF):
                    nc.tensor.matmul(ssq_q[:], ocol[:], sqq[c + QHALF][:],
                                     start=False, stop=(c == QHALF - 1))
                ms_q = tp.tile([1, SL], f32, name="ms_q", tag="ms", bufs=2)
                nc.scalar.activation(ms_q[:], ssq_q[:], AF.Sqrt, scale=1.0 / QLR)
                rq = tp.tile([1, SL], f32, name="rq", tag="rr", bufs=2)
                nc.vector.reciprocal_approx_fast(out=rq[:], in_=ms_q[:])
                r_bf = tp.tile([1, SL], dt, name="r_bf", tag="rbf", bufs=1)
                nc.vector.tensor_copy(r_bf[:], rq[:])
                nc.scalar.dma_start(out=ag_in_q[QCH * 128:QCH * 128 + 1, :], in_=r_bf[:])
                nc.gpsimd.collective_compute(
                    "AllGather", ALU.bypass,
                    replica_groups=[list(range(NC_))],
                    ins=[ag_in_q[:]], outs=[ag_out_q[:]],
                )

            agkv_r = ag_out_kv.rearrange("(r c) q -> r c q", r=NC_)
            agq_r = ag_out_q.rearrange("(r c) q -> r c q", r=NC_)

            # ================= Stage B =================
            with tc.tile_pool(name="sb_res", bufs=1) as sbp, \
                 tc.tile_pool(name="sb_tmp", bufs=2) as tp, \
                 tc.tile_pool(name="sb_qa", bufs=2) as qap, \
                 tc.tile_pool(name="sb_pt", bufs=4) as ptp, \
                 tc.tile_pool(name="sb_mm", bufs=2, space="PSUM") as pmm, \
                 tc.tile_pool(name="sb_at", bufs=2, space="PSUM") as pat, \
                 tc.tile_pool(name="sb_ps1", bufs=1, space="PSUM") as pp1:

                # K^T and V (both heads)
                kpe_g = sbp.tile([ROPE, S], dt, name="kpe_g")
                for r in range(NC_):
                    nc.sync.dma_start(out=kpe_g[:, SL * r:SL * (r + 1)],
                                      in_=agkv_r[r, KVLR:KVLR + ROPE, :])
                kT = [sbp.tile([128, S], dt, name=f"kT{h}") for h in range(HPC)]
                v_t = [sbp.tile([128, HPC * VD], dt, name=f"v_t{kb}") for kb in range(NKB)]
                with tc.tile_pool(name="sb_ckv", bufs=1) as ckvp:
                    ckv_g = []
                    for j in range(KCH):
                        t = ckvp.tile([128, S], dt, name=f"ckv_g{j}")
                        for r in range(NC_):
                            nc.sync.dma_start(out=t[:, SL * r:SL * (r + 1)],
                                              in_=agkv_r[r, 128 * j:128 * (j + 1), :])
                        ckv_g.append(t)
                    ei = 0
                    for h in range(HPC):
                        for kc in range(S // 512):
                            ps = pmm.tile([128, 512], f32, name=f"kt_ps{h}_{kc}", tag="mm", bufs=3)
                            for l in range(KCH):
                                nc.tensor.matmul(ps[:], wkk_t(l)[:, NOPE * h:NOPE * (h + 1)],
                                                 ckv_g[l][:, 512 * kc:512 * (kc + 1)],
                                                 start=(l == 0), stop=(l == KCH - 1))
                            if ei % 2 == 0:
                                nc.vector.tensor_copy(kT[h][:, 512 * kc:512 * (kc + 1)], ps[:])
                            else:
                                nc.scalar.activation(kT[h][:, 512 * kc:512 * (kc + 1)], ps[:], AF.Copy)
                            ei += 1
                    for kb in range(NKB):
                        ps = pmm.tile([128, HPC * VD], f32, name=f"v_ps{kb}", tag="mm", bufs=3)
                        for l in range(KCH):
                            nc.tensor.matmul(ps[:], ckv_g[l][:, 128 * kb:128 * (kb + 1)],
                                             wkv_t(l), start=(l == 0), stop=(l == KCH - 1))
                        if kb % 2 == 0:
                            nc.vector.tensor_copy(v_t[kb][:], ps[:])
                        else:
                            nc.scalar.activation(v_t[kb][:], ps[:], AF.Copy)

                # ---- q projections (single pass, rms scale applied here) ----
                qa_p = {}
                for p in range(NPANEL - 1, -1, -1):   # processing order
                    for l in range(QCH):
                        t = qap.tile([128, PANEL], dt, name=f"qa_p{p}_{l}", tag="qaa", bufs=16)
                        for r in range(2):
                            nc.sync.dma_start(out=t[:, SL * r:SL * (r + 1)],
                                              in_=agq_r[2 * p + r, 128 * l:128 * (l + 1), :])
                        qa_p[(p, l)] = t
                qn_sb = {}
                qp_sb = {}

                def qproj_panel(p):
                    qs = slice(PANEL * p, PANEL * (p + 1))
                    rrow = tp.tile([1, PANEL], dt, name=f"rrow{p}", tag="rrow", bufs=2)
                    for r in range(2):
                        nc.sync.dma_start(out=rrow[0:1, SL * r:SL * (r + 1)],
                                          in_=agq_r[2 * p + r, QCH * 128:QCH * 128 + 1, :])
                    rbc = pp1.tile([128, PANEL], f32, name=f"rbc{p}", tag="bcb", bufs=1)
                    nc.tensor.matmul(rbc[:], orow_bf[:], rrow[:], start=True, stop=True)
                    rbc_sb = tp.tile([128, PANEL], f32, name=f"rbc_sb{p}", tag="rbc_sb", bufs=2)
                    nc.scalar.activation(rbc_sb[:], rbc[:], AF.Copy)
                    for h in range(HPC):
                        hcol = 256 * h
                        ps_qn = pmm.tile([128, PANEL], f32, name=f"qn_ps{h}_{p}", tag="mm", bufs=3)
                        for l in range(QCH):
                            nc.tensor.matmul(ps_qn[:], wqb_t(l)[:, hcol:hcol + NOPE],
                                             qa_p[(p, l)][:], start=(l == 0), stop=(l == QCH - 1))
                        ps_qr = pmm.tile([128, PANEL], f32, name=f"qr_ps{h}_{p}", tag="mm", bufs=3)
                        for l in range(QCH):
                            nc.tensor.matmul(ps_qr[:], wqb_t(l)[:, hcol + NOPE:hcol + 256],
                                             qa_p[(p, l)][:], start=(l == 0), stop=(l == QCH - 1))
                        qn = sbp.tile([128, PANEL], dt, name=f"qn_sb{h}_{p}")
                        nc.vector.tensor_mul(qn[:], ps_qn[:], rbc_sb[:])
                        qn_sb[(h, p)] = qn
                        qt1 = tp.tile([ROPE, PANEL], f32, name=f"qt1_{h}_{p}", tag="qt1", bufs=2)
                        nc.vector.tensor_mul(qt1[:], ps_qr[0:ROPE, :], cos_all[:, qs])
                        qt2 = tp.tile([ROPE, PANEL], f32, name=f"qt2_{h}_{p}", tag="qt2", bufs=2)
                        nc.vector.tensor_mul(qt2[:], ps_qr[ROPE:2 * ROPE, :], sin_all[:, qs])
                        qpp = tp.tile([ROPE, PANEL], f32, name=f"qpp{h}_{p}", tag="qpp", bufs=2)
                        nc.vector.tensor_add(qpp[:], qt1[:], qt2[:])
                        qp = sbp.tile([ROPE, PANEL], dt, name=f"qp_sb{h}_{p}")
                        nc.vector.tensor_mul(qp[:], qpp[:], rbc_sb[0:ROPE, :])
                        qp_sb[(h, p)] = qp

                # ---- attention + per-panel Wo partials + AllToAll + reduce ----
                # heaviest panel first so its exchange hides under later panels
                PLIST = list(range(NPANEL - 1, -1, -1))
                with tc.tile_pool(name="sb_red", bufs=1) as redp:
                    def reduce_rows(srcs, dst_rows, tagsuf):
                        """f32 chain-reduce NC_ bf16 blocks, store to out_loc."""
                        acc = None
                        for r in range(1, NC_):
                            nxt = redp.tile(srcs[r].shape, f32, name=f"racc{tagsuf}_{r}",
                                            tag=f"racc{srcs[r].shape[1]}", bufs=2)
                            if acc is None:
                                nc.vector.tensor_add(nxt[:], srcs[0][:], srcs[1][:])
                            else:
                                nc.vector.tensor_add(nxt[:], acc[:], srcs[r][:])
                            acc = nxt
                        nc.sync.dma_start(out=dst_rows, in_=acc[:])

                    def reduce_panel(p):
                        rbs = []
                        for r in range(NC_):
                            t = redp.tile([SHARD, HID], dt, name=f"rb{p}_{r}", tag="rb", bufs=4)
                            nc.sync.dma_start(out=t[:], in_=a2a_out[p][SHARD * r:SHARD * (r + 1), :])
                            rbs.append(t)
                        reduce_rows(rbs, out_loc[SHARD * p:SHARD * (p + 1), :], f"p{p}")

                    for i, p in enumerate(PLIST):
                        last = (i == len(PLIST) - 1)
                        qproj_panel(p)
                        nkb = 4 * (p + 1)
                        at_ps = {}
                        ps_at = {h: pat.tile([128, PANEL], f32, name=f"at_ps{h}_{p}",
                                             tag="at", bufs=2) for h in range(HPC)}
                        ps_sum = {h: pp1.tile([1, PANEL], f32, name=f"sum_ps{h}_{p}",
                                              tag="sum", bufs=2) for h in range(HPC)}
                        pts = {h: {} for h in range(HPC)}

                        def consume(h, kb, nkb=nkb, ps_at=ps_at, ps_sum=ps_sum, pts=pts):
                            nc.tensor.matmul(ps_at[h][:], v_t[kb][:, VD * h:VD * (h + 1)],
                                             pts[h][kb][:],
                                             start=(kb == 0), stop=(kb == nkb - 1))
                            nc.tensor.matmul(ps_sum[h][:], ocol[:], pts[h][kb][:],
                                             start=(kb == 0), stop=(kb == nkb - 1))

                        # both heads interleaved per k-block: one head's exp/mask
                        # latency hides under the other head's matmuls
                        for kb in range(nkb):
                            for h in range(HPC):
                                ps_sc = pmm.tile([128, PANEL], f32, name=f"sc_ps{h}_{p}_{kb}",
                                                 tag="mm", bufs=3)
                                nc.tensor.matmul(ps_sc[:], kT[h][:, 128 * kb:128 * (kb + 1)],
                                                 qn_sb[(h, p)][:], start=True, stop=False)
                                nc.tensor.matmul(ps_sc[:], kpe_g[:, 128 * kb:128 * (kb + 1)],
                                                 qp_sb[(h, p)][:], start=False, stop=True)
                                pt = ptp.tile([128, PANEL], dt, name=f"pt{h}_{p}_{kb}",
                                              tag="pt", bufs=6)
                                if kb >= 4 * p:
                                    j = kb - 4 * p
                                    msc = tp.tile([128, PANEL], f32, name=f"msc{h}_{p}_{kb}",
                                                  tag="msc", bufs=3)
                                    nc.vector.tensor_add(msc[:], ps_sc[:],
                                                         mask_sb[:, PANEL * j:PANEL * (j + 1)])
                                    nc.scalar.activation(pt[:], msc[:], AF.Exp)
                                else:
                                    nc.scalar.activation(pt[:], ps_sc[:], AF.Exp)
                                pts[h][kb] = pt
                                if kb > 0:
                                    consume(h, kb - 1)
                        for h in range(HPC):
                            consume(h, nkb - 1)
                        for h in range(HPC):
                            rec = tp.tile([1, PANEL], f32, name=f"rec{h}_{p}", tag="rec", bufs=2)
                            nc.vector.reciprocal_approx_fast(out=rec[:], in_=ps_sum[h][:])
                            recr = tp.tile([1, PANEL], f32r, name=f"recr{h}_{p}", tag="recr", bufs=2)
                            with nc.allow_low_precision(reason="f32r rounding of softmax recip"):
                                nc.vector.tensor_copy(recr[:], rec[:])
                            bc = pp1.tile([128, PANEL], f32, name=f"bc_ps{h}_{p}", tag="bcb", bufs=1)
                            nc.tensor.matmul(bc[:], orow[:], recr[:], start=True, stop=True)
                            bc_sb = tp.tile([128, PANEL], f32, name=f"bc_sb{h}_{p}", tag="bc_sb", bufs=2)
                            nc.scalar.activation(bc_sb[:], bc[:], AF.Copy)
                            at_p = tp.tile([128, PANEL], dt, name=f"at_p{h}_{p}", tag="at_p", bufs=3)
                            nc.vector.tensor_mul(at_p[:], ps_at[h][:], bc_sb[:])
                            at_ps[h] = at_p

                        if not last:
                            # Wo partial, full hid width, one store per seq block
                            for sb in range(4):
                                ev = tp.tile([128, HID], dt, name=f"woev{p}_{sb}", tag="woev", bufs=2)
                                for n in range(4):
                                    ps_o = pmm.tile([128, 512], f32, name=f"wo_ps{p}_{sb}_{n}",
                                                    tag="mm", bufs=3)
                                    for h in range(HPC):
                                        nc.tensor.matmul(ps_o[:], at_ps[h][:, 128 * sb:128 * (sb + 1)],
                                                         wo_sb(h)[:, 512 * n:512 * (n + 1)],
                                                         start=(h == 0), stop=(h == HPC - 1))
                                    if sb % 2 == 0:
                                        nc.vector.tensor_copy(ev[:, 512 * n:512 * (n + 1)], ps_o[:])
                                    else:
                                        nc.scalar.activation(ev[:, 512 * n:512 * (n + 1)], ps_o[:],
                                                             AF.Copy)
                                nc.scalar.dma_start(out=a2a_in[p][128 * sb:128 * (sb + 1), :],
                                                    in_=ev[:])
                            nc.gpsimd.collective_compute(
                                "AllToAll", ALU.bypass,
                                replica_groups=[list(range(NC_))],
                                ins=[a2a_in[p][:]], outs=[a2a_out[p][:]],
                            )
                            if i > 0:
                                reduce_panel(PLIST[i - 1])
                        else:
                            # last processed (lightest) panel: exchange in two hid
                            # halves so the first half's reduce overlaps the second
                            prev_done = False
                            for half in range(2):
                                hid0 = (HID // 2) * half
                                for sb in range(4):
                                    ev = tp.tile([128, HID // 2], dt, name=f"wol{half}_{sb}",
                                                 tag="wolev", bufs=2)
                                    for n in range(2):
                                        ps_o = pmm.tile([128, 512], f32, name=f"wol_ps{half}_{sb}_{n}",
                                                        tag="mm", bufs=3)
                                        for h in range(HPC):
                                            nc.tensor.matmul(
                                                ps_o[:], at_ps[h][:, 128 * sb:128 * (sb + 1)],
                                                wo_sb(h)[:, hid0 + 512 * n:hid0 + 512 * (n + 1)],
                                                start=(h == 0), stop=(h == HPC - 1))
                                        if sb % 2 == 0:
                                            nc.vector.tensor_copy(ev[:, 512 * n:512 * (n + 1)], ps_o[:])
                                        else:
                                            nc.scalar.activation(ev[:, 512 * n:512 * (n + 1)],
                                                                 ps_o[:], AF.Copy)
                                    nc.scalar.dma_start(out=a2a_lin[half][128 * sb:128 * (sb + 1), :],
                                                        in_=ev[:])
                                nc.gpsimd.collective_compute(
                                    "AllToAll", ALU.bypass,
                                    replica_groups=[list(range(NC_))],
                                    ins=[a2a_lin[half][:]], outs=[a2a_lout[half][:]],
                                )
                                if not prev_done and i > 0:
                                    prev_done = True
                                    reduce_panel(PLIST[i - 1])
                            for half in range(2):
                                hid0 = (HID // 2) * half
                                rbs = []
                                for r in range(NC_):
                                    t = redp.tile([SHARD, HID // 2], dt, name=f"rbl{half}_{r}",
                                                  tag="rbl", bufs=4)
                                    nc.sync.dma_start(out=t[:],
                                                      in_=a2a_lout[half][SHARD * r:SHARD * (r + 1), :])
                                    rbs.append(t)
                                reduce_rows(rbs,
                                            out_loc[SHARD * p:SHARD * (p + 1), hid0:hid0 + HID // 2],
                                            f"l{half}")

    nc.compile()
    return nc


def _to_dt(a, dt):
    if dt == bf16:
        return np.ascontiguousarray(a.astype(ml_dtypes.bfloat16))
    return np.ascontiguousarray(a.astype(np.float32))


def _prepare_inputs(dt, hidden_states, position_ids, Wqa, qa_ln_w, Wqb, Wkva, kv_ln_w, Wkvb, Wo):
    perm = np.concatenate([np.arange(0, ROPE, 2), np.arange(1, ROPE, 2)])
    X = np.asarray(hidden_states, np.float32).reshape(S, HID)
    pos_f = np.ascontiguousarray(np.asarray(position_ids, np.float32).reshape(1, S))
    Wqa = np.asarray(Wqa, np.float32)
    Wkva = np.asarray(Wkva, np.float32)
    wa_kv = np.concatenate([Wkva[:, :KVLR], Wkva[:, KVLR:][:, perm]], axis=1)  # (2048, 576)
    wqb_base = np.asarray(Wqb, np.float32) * np.asarray(qa_ln_w, np.float32)[:, None]
    wkvb_base = np.asarray(Wkvb, np.float32) * np.asarray(kv_ln_w, np.float32)[:, None]
    Wo = np.asarray(Wo, np.float32)

    head_blocks = []
    for h in range(NH):
        cols = wqb_base[:, 192 * h:192 * (h + 1)] * SM_SCALE
        nope = cols[:, :NOPE]
        pe_d = cols[:, NOPE:][:, perm]
        rot = np.concatenate([-pe_d[:, 32:], pe_d[:, :32]], axis=1)
        head_blocks.append(np.concatenate([nope, pe_d, rot], axis=1))  # (1536, 256)
    k_blocks = [wkvb_base[:, 256 * h:256 * h + NOPE] for h in range(NH)]
    v_blocks = [wkvb_base[:, 256 * h + NOPE:256 * (h + 1)] for h in range(NH)]

    inv = (1.0 / (THETA ** (np.arange(0, ROPE, 2, dtype=np.float32) / ROPE))).astype(np.float32)
    invf_np = np.concatenate([inv, inv])

    # diagonal masks: block j, mask[r, col] = NEG where col < 128 j + r
    colsi = np.arange(PANEL)[None, :]
    rowsi = np.arange(128)[:, None]
    mask_np = np.concatenate(
        [np.where(colsi < 128 * j + rowsi, NEG, 0.0) for j in range(4)], axis=1
    ).astype(np.float32)

    def pack_rows(w):
        ch = w.shape[0] // 128
        return np.concatenate([w[128 * k:128 * (k + 1), :] for k in range(ch)], axis=1)

    wa_kv_d = _to_dt(pack_rows(wa_kv), dt)
    wa_q_d = _to_dt(pack_rows(Wqa), dt)
    mask_d = _to_dt(mask_np, dt)
    ones_col_d = _to_dt(np.ones((128, 1), np.float32), dt)

    in_maps = []
    for c in range(NC_):
        rows_c = slice(SL * c, SL * (c + 1))
        wqb_c = np.concatenate([head_blocks[HPC * c + h] for h in range(HPC)], axis=1)
        wkk_c = np.concatenate([k_blocks[HPC * c + h] for h in range(HPC)], axis=1)
        wkv_c = np.concatenate([v_blocks[HPC * c + h] for h in range(HPC)], axis=1)
        wo_c = np.concatenate([Wo[VD * (HPC * c + h):VD * (HPC * c + h + 1), :]
                               for h in range(HPC)], axis=0)
        in_maps.append({
            "x_p": _to_dt(pack_rows(np.ascontiguousarray(X[rows_c, :].T)), dt),
            "pos": np.ascontiguousarray(pos_f[:, rows_c]),
            "pos_all": pos_f,
            "wakv_p": wa_kv_d,
            "waq_p": wa_q_d,
            "wqb_p": _to_dt(pack_rows(wqb_c), dt),
            "wkk_p": _to_dt(pack_rows(wkk_c), dt),
            "wkv_p": _to_dt(pack_rows(wkv_c), dt),
            "wo_p": _to_dt(pack_rows(wo_c), dt),
            "mask": mask_d,
            "ones_col": ones_col_d,
            "ones_row": np.ones((1, 128), np.float32),
            "invf_col": invf_np.reshape(ROPE, 1).copy(),
        })
    return in_maps


def run(inputs, trace=False, trace_cores=None, dt=None):
    dt = dt if dt is not None else DT
    key = ("nc", str(dt))
    if key not in _CACHE:
        _CACHE[key] = build_program(dt)
    nc = _CACHE[key]
    in_maps = _prepare_inputs(dt, **inputs)
    res = run_bass_kernel_spmd(nc, in_maps, list(range(NC_)), trace=trace,
                               trace_cores=trace_cores)
    # reassemble: panel p, core c holds global seq rows [512 p + 64 c, 512 p + 64 (c+1))
    out = np.empty((S, HID), np.float32)
    for c in range(NC_):
        o = res.results[c]["out_loc"]
        for p in range(NPANEL):
            out[PANEL * p + SHARD * c:PANEL * p + SHARD * (c + 1), :] = \
                o[SHARD * p:SHARD * (p + 1), :]
    return out.reshape(1, S, HID), res


def kernel(**inputs) -> np.ndarray:
    out, _ = run(inputs, trace=False)
    return out
